# revision 2
# baseline (speedup 1.0000x reference)
"""Bahdanau additive attention (causal) on 8 TRN2 cores — v2.

Per core (batch b, query-parity h): 256 strided queries i (qs[i]=2i+h),
512 keys j.  score^T layout: regions sc[jt] = [128 keys j, 256 queries i]
in PSUM, computed as sum_u vfeat[u, j-tile] x qfold[u, i] matmuls, so the
exp output esc[jt] = [j, i] is DIRECTLY the ctx matmul's lhsT:
    ctx[i, d] = sum_j esc[j, i] * values[j, d]
-> no attention transposes / copies at all. The values rhs carries a 257th
ones-column so each ctx psum tile's last column accumulates ssum_i for
free. Softmax normalization (and the query mask) is applied per-partition
to the final [i, d] psum tiles.

tanh(x) ~= sum_k b_k sin(nu_k x) (K harmonics, minimax fit on |x|<=8.8,
tail-weighted). Feature args are range-reduced in revolutions via the f16
magic-rounding trick (z, u=z+1536, n=u-1536, r=n-z, a=|r|) on DVE in
4x/2x perf-mode forms; sin/cos features on ACT (Sin activation); the
b_k*Vw_u fold rides per-(ut,k) dual tensor_scalar ops with a per-partition
f32 scalar AP (4x) instead of a tensor-tensor with a materialized fold
tile. Causal+key mask initializes each score region via an identity
matmul; score matmuls cover only the causal column extent [64*jt, 256).
Constant exp shift -4 replaces the row-max pass."""

import sys

sys.path.insert(0, "/opt/trn_rl_repo")

import numpy as np

import concourse.bass as bass
import concourse.bacc as bacc
import concourse.tile as tile
from concourse import mybir
from concourse.bass_utils import run_bass_kernel_spmd

B, S, D, U = 4, 512, 256, 256
N_CORES = 8
NEG16 = -30000.0

f32 = mybir.dt.float32
f16 = mybir.dt.float16
u16 = mybir.dt.uint16
AF = mybir.ActivationFunctionType
OP = mybir.AluOpType

# Minimax fits of tanh on [0, 8.81], tail-weighted (x>6.5 weight 0.25).
# K=5: fit maxerr 8.0e-3, simulated end-to-end rel err 4.4e-3 (HW 4.4e-3).
# K=4: fit maxerr 1.39e-2, simulated end-to-end rel err 1.05e-2.
FITS = {
    5: ([0.300242, 0.906507, 1.525431, 2.171803, 2.850332],
        [1.2297972, 0.3126199, 0.1147746, 0.0450979, 0.0178077]),
    4: ([0.30879, 0.933012, 1.588643, 2.298548],
        [1.2262237, 0.3095406, 0.1132022, 0.0440302]),
}
OM, BK = FITS[5]
K = len(OM)
TWO_PI = 2.0 * np.pi
PI = np.pi
MAGIC = 1536.0
SHIFT = -4.0
DEBUG = False


def _build_program():
    nc = bacc.Bacc("TRN2", target_bir_lowering=False, debug=False)

    # packed inputs (dt/jt-major along free dim so each is ONE dma)
    wv_ap = nc.dram_tensor("wv", [128, 2 * 256], f16, kind="ExternalInput").ap()
    wq_ap = nc.dram_tensor("wq", [128, 2 * 256], f16, kind="ExternalInput").ap()
    vT_ap = nc.dram_tensor("vT", [128, 2 * 512], f16, kind="ExternalInput").ap()
    qT_ap = nc.dram_tensor("qT", [128, 2 * 256], f16, kind="ExternalInput").ap()
    val_ap = nc.dram_tensor("val", [128, 4 * 257], f16, kind="ExternalInput").ap()
    cau_ap = nc.dram_tensor("cau", [128, 4 * 256], f16, kind="ExternalInput").ap()
    ident_ap = nc.dram_tensor("ident", [128, 128], f16, kind="ExternalInput").ap()
    # fold scalars: col ut*K+k = b_k * Vw[128*ut:128*(ut+1)]; col 2K = qmask
    fold_ap = nc.dram_tensor("fold", [128, 2 * K + 2], f32,
                             kind="ExternalInput").ap()
    ctx_ap = nc.dram_tensor("ctx", [256, D], f16, kind="ExternalOutput").ap()
    dbg_aps = {}
    if DEBUG:
        for nm, shape, dt in [("d_pjv0", [128, 512], f16),
                              ("d_pjq0", [128, 256], f16),
                              ("d_sfq0", [128, K * 256], f16),
                              ("d_cfv0", [128, K * 512], f16),
                              ("d_qws0", [128, K * 256], f16),
                              ("d_esc0", [128, 256], f16),
                              ("d_esc3", [128, 256], f16),
                              ("d_ctx0", [128, 257], f32),
                              ("d_init0", [128, 256], f32)]:
            dbg_aps[nm] = nc.dram_tensor(nm, shape, dt, kind="ExternalOutput").ap()

    from contextlib import ExitStack

    with tile.TileContext(nc) as tc, ExitStack() as es:
        const = es.enter_context(tc.tile_pool(name="const", bufs=1))
        work = es.enter_context(tc.tile_pool(name="work", bufs=1))
        spool = es.enter_context(tc.tile_pool(name="small", bufs=4))
        pp = es.enter_context(tc.tile_pool(name="psum", bufs=1, space="PSUM"))

        # ---- consts on DVE (idle until ~4us; Pool's SWDGE gens must not
        # delay ones16 -> pewarm -> the PE p-state ramp)
        ones16 = const.tile([1, 128], f16, tag="ones16")
        nc.vector.memset(ones16[:], 1.0)
        bias_z = const.tile([128, 1], f32, tag="bz")
        nc.vector.memset(bias_z[:], 0.0)
        dummy = const.tile([1, 128], f16, tag="dummy")
        nc.vector.memset(dummy[:], 0.25)
        bias_hpi = const.tile([128, 1], f32, tag="bhpi")
        nc.vector.memset(bias_hpi[:], PI / 2)
        bias_sh = const.tile([128, 1], f32, tag="bsh")
        nc.vector.memset(bias_sh[:], SHIFT)
        nc.scalar.activation(dummy[:], dummy[:], AF.Sin, bias=bias_z[0:1, :])

        # ---- input DMAs. HWDGE (sync/scalar issue) gens serialize at 625ns
        # on one device; Pool SWDGE is a parallel generator (~1us/dma on the
        # Pool engine). v-projection operands first (v chain is longest).
        vT_sb = work.tile([128, 2 * 512], f16, tag="vT")
        nc.sync.dma_start(vT_sb[:], vT_ap)
        wq_sb = work.tile([128, 2 * 256], f16, tag="wq")
        nc.scalar.dma_start(wq_sb[:], wq_ap)
        fold_sb = const.tile([128, 2 * K + 2], f32, tag="fold")
        nc.sync.dma_start(fold_sb[:], fold_ap)
        ident_sb = const.tile([128, 128], f16, tag="ident")
        nc.scalar.dma_start(ident_sb[:], ident_ap)
        wv_sb = work.tile([128, 2 * 256], f16, tag="wv")
        nc.gpsimd.dma_start(wv_sb[:], wv_ap)
        qT_sb = work.tile([128, 2 * 256], f16, tag="qT")
        nc.gpsimd.dma_start(qT_sb[:], qT_ap)
        cau_sb = const.tile([128, 4 * 256], f16, tag="cau")
        nc.gpsimd.dma_start(cau_sb[:], cau_ap)
        val_sb = work.tile([128, 4 * 257], f16, tag="val")
        nc.gpsimd.dma_start(val_sb[:], val_ap)
        # PSUM: one accumulation group per BANK at a time (start=True
        # invalidates the whole bank). 6 banks: psv0 psv1 (512f32 = 1 bank
        # each), psq0 psq1, ctx0 ctx1. Score regions REUSE the projection
        # banks (projection groups are stopped and fully read by the DVE
        # converts before each score init; the WAR dep rides the AP overlap).
        psq_t = [pp.tile([128, 256], f32, tag=f"psq{ut}", name=f"psq{ut}")
                 for ut in range(2)]

        # ---- projections -> PSUM f32, then DVE converts to f16
        # psv[ut]: [128 u, 512 j], psq[ut]: [128 u, 256 i]
        psv, psq = [], []
        for ut in range(2):
            ps = pp.tile([128, 512], f32, tag=f"psv{ut}", name=f"psv{ut}")
            if ut == 0:
                # PE p-state warm-up rides a corner of psv0 (wiped by the
                # projection's own start=True; nothing reads it)
                nc.tensor.matmul(
                    ps[:, 0:1], lhsT=ones16[:], rhs=ones16[:, 0:1],
                    start=True, stop=True, skip_group_check=True,
                )
            for dt in range(2):
                nc.tensor.matmul(
                    ps[:],
                    lhsT=wv_sb[:, dt * 256 + ut * 128: dt * 256 + ut * 128 + 128],
                    rhs=vT_sb[:, dt * 512: dt * 512 + 512],
                    start=(dt == 0), stop=(dt == 1),
                )
            psv.append(ps)
        for ut in range(2):
            ps = psq_t[ut]
            for dt in range(2):
                nc.tensor.matmul(
                    ps[:],
                    lhsT=wq_sb[:, dt * 256 + ut * 128: dt * 256 + ut * 128 + 128],
                    rhs=qT_sb[:, dt * 256: dt * 256 + 256],
                    start=(dt == 0), stop=(dt == 1),
                )
            psq.append(ps[:])

        projs = {}
        conv_src = {("v", 0): psv[0][:], ("v", 1): psv[1][:],
                    ("q", 0): psq[0], ("q", 1): psq[1]}
        for side, ut, Wd in (("v", 0, 512), ("q", 0, 256),
                             ("v", 1, 512), ("q", 1, 256)):
            projs[(side, ut)] = work.tile(
                [128, Wd], f16, tag=f"pj{side}{ut}", name=f"pj{side}{ut}")

        def emit_conv(side, ut):
            nc.vector.tensor_copy(projs[(side, ut)][:], conv_src[(side, ut)])

        # ---- arg chains (DVE) / features (ACT) / folds (DVE) / scores (PE)
        streams = [("v", 0, 512), ("q", 0, 256), ("v", 1, 512), ("q", 1, 256)]
        r_t, a_t, z_t, u_t, n_t, s_f, c_f = {}, {}, {}, {}, {}, {}, {}
        for side, ut, Wd in streams:
            key = (side, ut)
            r_t[key] = work.tile([128, K * Wd], f16, tag=f"r{side}{ut}", name=f"r{side}{ut}")
            a_t[key] = work.tile([128, K * Wd], f16, tag=f"a{side}{ut}", name=f"a{side}{ut}")
            z_t[key] = work.tile([128, (K - 1) * Wd], f16, tag=f"z{side}{ut}", name=f"z{side}{ut}")
            u_t[key] = work.tile([128, (K - 1) * Wd], f16, tag=f"u{side}{ut}", name=f"u{side}{ut}")
            n_t[key] = work.tile([128, (K - 1) * Wd], f16, tag=f"n{side}{ut}", name=f"n{side}{ut}")
            s_f[key] = work.tile([128, K * Wd], f16, tag=f"s{side}{ut}", name=f"s{side}{ut}")
            c_f[key] = work.tile([128, K * Wd], f16, tag=f"c{side}{ut}", name=f"c{side}{ut}")
        qws = [work.tile([128, K * 256], f16, tag=f"qws{ut}", name=f"qws{ut}") for ut in range(2)]
        qwc = [work.tile([128, K * 256], f16, tag=f"qwc{ut}", name=f"qwc{ut}") for ut in range(2)]

        # chunks of k-space: k0 alone unblocks ACT/PE early; rest split so
        # the tail releases score matmuls finely.
        CHUNKS = [(0, 1), (1, 3), (3, K)]

        def emit_args(side, ut, c0, c1):
            key = (side, ut)
            Wd = 512 if side == "v" else 256
            proj = projs[key]
            r, a, z, u, n = r_t[key], a_t[key], z_t[key], u_t[key], n_t[key]
            for k in range(c0, c1):
                if k == 0:
                    nc.vector.tensor_scalar_mul(
                        r[:, 0:Wd], proj[:], float(-OM[0] / TWO_PI))
                else:
                    nc.vector.tensor_scalar_mul(
                        z[:, (k - 1) * Wd: k * Wd], proj[:],
                        float(OM[k] / TWO_PI))
            z0, z1 = max(c0 - 1, 0), c1 - 1
            if z1 > z0:
                zs = slice(z0 * Wd, z1 * Wd)
                nc.vector.tensor_scalar_add(u[:, zs], z[:, zs], MAGIC)
                nc.vector.tensor_scalar_sub(n[:, zs], u[:, zs], MAGIC)
                nc.vector.tensor_tensor(
                    r[:, (z0 + 1) * Wd: (z1 + 1) * Wd], n[:, zs], z[:, zs],
                    op=OP.subtract)
            nc.vector.tensor_scalar(
                a[:, c0 * Wd: c1 * Wd].bitcast(u16),
                r[:, c0 * Wd: c1 * Wd].bitcast(u16),
                0x7FFF, None, op0=OP.bitwise_and)

        def emit_feats(side, ut, c0, c1):
            key = (side, ut)
            Wd = 512 if side == "v" else 256
            cs = slice(c0 * Wd, c1 * Wd)
            # r ~= n - z  ->  sin(2pi z) = Sin(-2pi r);  cos = Sin(-2pi|r|+pi/2)
            nc.scalar.activation(s_f[key][:, cs], r_t[key][:, cs], AF.Sin,
                                 scale=-TWO_PI, bias=bias_z[:])
            nc.scalar.activation(c_f[key][:, cs], a_t[key][:, cs], AF.Sin,
                                 scale=-TWO_PI, bias=bias_hpi[:])

        def emit_folds(ut, c0, c1):
            for k in range(c0, c1):
                cs = slice(k * 256, (k + 1) * 256)
                col = fold_sb[:, ut * K + k: ut * K + k + 1]
                nc.vector.tensor_scalar(qws[ut][:, cs], s_f[("q", ut)][:, cs],
                                        col, None, op0=OP.mult)
                nc.vector.tensor_scalar(qwc[ut][:, cs], c_f[("q", ut)][:, cs],
                                        col, None, op0=OP.mult)

        # score regions sc[jt]: [128 j, 256 i]; mask-init; extent i0=64*jt
        sc_banks = [psv[0], psv[1], psq_t[0], psq_t[1]]

        def sc_slice(jt, i0, i1):
            return sc_banks[jt][:, i0:i1]

        def emit_init(jt):
            nc.tensor.matmul(
                sc_slice(jt, 0, 256), lhsT=ident_sb[:],
                rhs=cau_sb[:, jt * 256: jt * 256 + 256],
                start=True, stop=False, skip_group_check=True)
        if DEBUG:
            d_init = spool.tile([128, 256], f32, tag="dinit", name="d_init")
            nc.vector.tensor_copy(d_init[:], sc_slice(0, 0, 256))
            nc.sync.dma_start(dbg_aps["d_init0"], d_init[:])
        reg_left = [2 * 2 * K] * 4

        def emit_scores(ut, c0, c1):
            for k in range(c0, c1):
                for jt in range(4):
                    i0 = 64 * jt
                    for lhs_f, rhs_t in ((c_f[("v", ut)], qws[ut]),
                                         (s_f[("v", ut)], qwc[ut])):
                        reg_left[jt] -= 1
                        nc.tensor.matmul(
                            sc_slice(jt, i0, 256),
                            lhsT=lhs_f[:, k * 512 + jt * 128: k * 512 + jt * 128 + 128],
                            rhs=rhs_t[:, k * 256 + i0: k * 256 + 256],
                            start=False, stop=(reg_left[jt] == 0),
                            skip_group_check=True)

        # esc tiles + zero-fill of non-causal columns
        esc = []
        for jt in range(4):
            t = work.tile([128, 256], f16, tag=f"esc{jt}", name=f"esc{jt}")
            if jt > 0:
                nc.gpsimd.memset(t[:, 0: 64 * jt], 0.0)
            esc.append(t)

        # pipeline, ordered per-engine so DVE stays ahead of ACT and PE
        emit_conv("v", 0)
        emit_args("v", 0, 0, 1)
        emit_feats("v", 0, 0, 1)
        emit_init(0)
        emit_conv("q", 0)
        emit_args("q", 0, 0, 1)
        emit_feats("q", 0, 0, 1)
        emit_init(2)
        emit_conv("v", 1)
        emit_args("v", 1, 0, 1)
        emit_feats("v", 1, 0, 1)
        emit_init(1)
        emit_conv("q", 1)
        emit_args("q", 1, 0, 1)
        emit_feats("q", 1, 0, 1)
        emit_init(3)
        emit_folds(0, 0, 1)
        emit_scores(0, 0, 1)
        emit_folds(1, 0, 1)
        emit_scores(1, 0, 1)
        emit_args("v", 0, 1, 3)
        emit_feats("v", 0, 1, 3)
        emit_args("q", 0, 1, K)
        emit_feats("q", 0, 1, K)
        emit_args("v", 1, 1, 3)
        emit_feats("v", 1, 1, 3)
        emit_folds(0, 1, 3)
        emit_scores(0, 1, 3)
        emit_args("q", 1, 1, K)
        emit_feats("q", 1, 1, K)
        emit_folds(1, 1, 3)
        emit_scores(1, 1, 3)
        emit_args("v", 0, 3, K)
        emit_feats("v", 0, 3, K)
        emit_folds(0, 3, K)
        emit_scores(0, 3, K)
        emit_args("v", 1, 3, K)
        emit_feats("v", 1, 3, K)
        emit_folds(1, 3, K)
        emit_scores(1, 3, K)

        # softmax + context
        ctx_ps = []
        for it in range(2):
            t = pp.tile([128, 257], f32, tag=f"ctx{it}", name=f"ctx{it}")
            ctx_ps.append(t)
        for jt in range(4):
            i0 = 64 * jt
            nc.scalar.activation(esc[jt][:, i0:256], sc_slice(jt, i0, 256),
                                 AF.Exp, bias=bias_sh[:])
            for it in range(2):
                nc.tensor.matmul(
                    ctx_ps[it][:],
                    lhsT=esc[jt][:, it * 128: it * 128 + 128],
                    rhs=val_sb[:, jt * 257: jt * 257 + 257],
                    start=(jt == 0), stop=(jt == 3),
                )
        if DEBUG:
            nc.sync.dma_start(dbg_aps["d_pjv0"], projs[("v", 0)][:])
            nc.sync.dma_start(dbg_aps["d_pjq0"], projs[("q", 0)][:])
            nc.sync.dma_start(dbg_aps["d_sfq0"], s_f[("q", 0)][:])
            nc.sync.dma_start(dbg_aps["d_cfv0"], c_f[("v", 0)][:])
            nc.sync.dma_start(dbg_aps["d_qws0"], qws[0][:])
            nc.sync.dma_start(dbg_aps["d_esc0"], esc[0][:])
            nc.sync.dma_start(dbg_aps["d_esc3"], esc[3][:])
            dbg_ctx = spool.tile([128, 257], f32, tag="dbgctx", name="dbgctx")
            nc.vector.tensor_copy(dbg_ctx[:], ctx_ps[0][:])
            nc.sync.dma_start(dbg_aps["d_ctx0"], dbg_ctx[:])
        for it in range(2):
            rcp = spool.tile([128, 1], f32, tag="rcp", name=f"rcp{it}")
            nc.vector.reciprocal(rcp[:], ctx_ps[it][:, 256:257])
            rq = spool.tile([128, 1], f32, tag="rq", name=f"rq{it}")
            nc.vector.tensor_scalar(rq[:], rcp[:],
                                    fold_sb[:, 2 * K + it: 2 * K + it + 1],
                                    None, op0=OP.mult)
            octx = spool.tile([128, 256], f16, tag="octx", name=f"octx{it}")
            nc.vector.tensor_scalar(octx[:], ctx_ps[it][:, 0:256],
                                    rq[:, 0:1], None, op0=OP.mult)
            (nc.sync if it == 0 else nc.scalar).dma_start(
                ctx_ap[it * 128: it * 128 + 128, :], octx[:])

    nc.compile()
    return nc


_NC_CACHE = {}


def _get_nc():
    if "nc" not in _NC_CACHE:
        _NC_CACHE["nc"] = _build_program()
    return _NC_CACHE["nc"]


def _qsel(h):
    return np.arange(h, S, 2)


def build_in_maps(values, mask, Wq, Wv, Vw):
    values = np.asarray(values, dtype=np.float32)
    mask = np.asarray(mask)
    Wq = np.asarray(Wq, dtype=np.float32)
    Wv = np.asarray(Wv, dtype=np.float32)
    Vw = np.asarray(Vw, dtype=np.float32)

    # weights packed dt-major: [128, 2*256], col block dt -> d rows
    wv_p = np.concatenate([Wv[0:128, :], Wv[128:256, :]], axis=1).astype(np.float16)
    wq_p = np.concatenate([Wq[0:128, :], Wq[128:256, :]], axis=1).astype(np.float16)

    jcol = np.arange(S)
    in_maps = []
    for c in range(N_CORES):
        b, h = divmod(c, 2)
        qs = _qsel(h)
        vb = values[b]  # [512, 256]
        vT = vb.T.astype(np.float16)  # [256 d, 512 j]
        vT_p = np.concatenate([vT[0:128, :], vT[128:256, :]], axis=1)
        qT = vb[qs].T.astype(np.float16)  # [256 d, 256 i]
        qT_p = np.concatenate([qT[0:128, :], qT[128:256, :]], axis=1)
        # values+ones packed jt-major: [128, 4*257]
        vo = np.concatenate(
            [vb.astype(np.float16), np.ones((S, 1), np.float16)], axis=1)
        val_p = np.concatenate([vo[128 * jt: 128 * (jt + 1), :]
                                for jt in range(4)], axis=1)
        # causalT[j, i]: invalid iff qs[i] < j or key j masked out
        inval = (qs[None, :] < jcol[:, None]) | (~mask[b])[:, None]
        cauT = (inval * NEG16).astype(np.float16)  # [512 j, 256 i]
        cau_p = np.concatenate([cauT[128 * jt: 128 * (jt + 1), :]
                                for jt in range(4)], axis=1)
        # fold scalars + per-it query mask columns
        fold = np.zeros((128, 2 * K + 2), np.float32)
        for ut in range(2):
            for k in range(K):
                fold[:, ut * K + k] = BK[k] * Vw[128 * ut: 128 * (ut + 1)]
        qm = mask[b][qs].astype(np.float32)
        fold[:, 2 * K] = qm[0:128]
        fold[:, 2 * K + 1] = qm[128:256]
        in_maps.append({
            "wv": wv_p, "wq": wq_p,
            "vT": np.ascontiguousarray(vT_p),
            "qT": np.ascontiguousarray(qT_p),
            "val": np.ascontiguousarray(val_p),
            "cau": np.ascontiguousarray(cau_p),
            "ident": np.eye(128, dtype=np.float16),
            "fold": fold,
        })
    return in_maps


def kernel(values, mask, Wq, Wv, Vw):
    nc = _get_nc()
    in_maps = build_in_maps(values, mask, Wq, Wv, Vw)
    res = run_bass_kernel_spmd(nc, in_maps, list(range(N_CORES)))

    out = np.empty((B, S, D), dtype=np.float32)
    for c in range(N_CORES):
        b, h = divmod(c, 2)
        out[b, _qsel(h)] = res.results[c]["ctx"].astype(np.float32)
    return out


# revision 4
# speedup vs baseline: 1.0013x; 1.0013x over previous
"""Bahdanau additive attention (causal) on 8 TRN2 cores — v2.

Per core (batch b, query-parity h): 256 strided queries i (qs[i]=2i+h),
512 keys j.  score^T layout: regions sc[jt] = [128 keys j, 256 queries i]
in PSUM, computed as sum_u vfeat[u, j-tile] x qfold[u, i] matmuls, so the
exp output esc[jt] = [j, i] is DIRECTLY the ctx matmul's lhsT:
    ctx[i, d] = sum_j esc[j, i] * values[j, d]
-> no attention transposes / copies at all. The values rhs carries a 257th
ones-column so each ctx psum tile's last column accumulates ssum_i for
free. Softmax normalization (and the query mask) is applied per-partition
to the final [i, d] psum tiles.

tanh(x) ~= sum_k b_k sin(nu_k x) (K harmonics, minimax fit on |x|<=8.8,
tail-weighted). Feature args are range-reduced in revolutions via the f16
magic-rounding trick (z, u=z+1536, n=u-1536, r=n-z, a=|r|) on DVE in
4x/2x perf-mode forms; sin/cos features on ACT (Sin activation); the
b_k*Vw_u fold rides per-(ut,k) dual tensor_scalar ops with a per-partition
f32 scalar AP (4x) instead of a tensor-tensor with a materialized fold
tile. Causal+key mask initializes each score region via an identity
matmul; score matmuls cover only the causal column extent [64*jt, 256).
Constant exp shift -4 replaces the row-max pass."""

import sys

sys.path.insert(0, "/opt/trn_rl_repo")

import numpy as np

import concourse.bass as bass
import concourse.bacc as bacc
import concourse.tile as tile
from concourse import mybir
from concourse.bass_utils import run_bass_kernel_spmd

B, S, D, U = 4, 512, 256, 256
N_CORES = 8
NEG16 = -30000.0

f32 = mybir.dt.float32
f16 = mybir.dt.float16
u16 = mybir.dt.uint16
AF = mybir.ActivationFunctionType
OP = mybir.AluOpType

# Minimax fits of tanh on [0, 8.81], tail-weighted (x>6.5 weight 0.25).
# K=5: fit maxerr 8.0e-3, simulated end-to-end rel err 4.4e-3 (HW 4.4e-3).
# K=4: fit maxerr 1.39e-2, simulated end-to-end rel err 1.05e-2.
FITS = {
    5: ([0.300242, 0.906507, 1.525431, 2.171803, 2.850332],
        [1.2297972, 0.3126199, 0.1147746, 0.0450979, 0.0178077]),
    4: ([0.30879, 0.933012, 1.588643, 2.298548],
        [1.2262237, 0.3095406, 0.1132022, 0.0440302]),
}
OM, BK = FITS[5]
K = len(OM)
TWO_PI = 2.0 * np.pi
PI = np.pi
MAGIC = 1536.0
SHIFT = -4.0
DEBUG = False


def _build_program():
    nc = bacc.Bacc("TRN2", target_bir_lowering=False, debug=False)

    # packed inputs (dt/jt-major along free dim so each is ONE dma)
    wv_ap = nc.dram_tensor("wv", [128, 2 * 256], f16, kind="ExternalInput").ap()
    wq_ap = nc.dram_tensor("wq", [128, 2 * 256], f16, kind="ExternalInput").ap()
    vT_ap = nc.dram_tensor("vT", [128, 2 * 512], f16, kind="ExternalInput").ap()
    qT_ap = nc.dram_tensor("qT", [128, 2 * 256], f16, kind="ExternalInput").ap()
    val_ap = nc.dram_tensor("val", [128, 4 * 257], f16, kind="ExternalInput").ap()
    cau_ap = nc.dram_tensor("cau", [128, 4 * 256], f16, kind="ExternalInput").ap()
    ident_ap = nc.dram_tensor("ident", [128, 128], f16, kind="ExternalInput").ap()
    # fold scalars: col ut*K+k = b_k * Vw[128*ut:128*(ut+1)]; col 2K = qmask
    fold_ap = nc.dram_tensor("fold", [128, 2 * K + 2], f32,
                             kind="ExternalInput").ap()
    ctx_ap = nc.dram_tensor("ctx", [256, D], f16, kind="ExternalOutput").ap()
    dbg_aps = {}
    if DEBUG:
        for nm, shape, dt in [("d_pjv0", [128, 512], f16),
                              ("d_pjq0", [128, 256], f16),
                              ("d_sfq0", [128, K * 256], f16),
                              ("d_cfv0", [128, K * 512], f16),
                              ("d_qws0", [128, K * 256], f16),
                              ("d_esc0", [128, 256], f16),
                              ("d_esc3", [128, 256], f16),
                              ("d_ctx0", [128, 257], f32),
                              ("d_init0", [128, 256], f32)]:
            dbg_aps[nm] = nc.dram_tensor(nm, shape, dt, kind="ExternalOutput").ap()

    from contextlib import ExitStack

    with tile.TileContext(nc) as tc, ExitStack() as es:
        const = es.enter_context(tc.tile_pool(name="const", bufs=1))
        work = es.enter_context(tc.tile_pool(name="work", bufs=1))
        spool = es.enter_context(tc.tile_pool(name="small", bufs=4))
        pp = es.enter_context(tc.tile_pool(name="psum", bufs=1, space="PSUM"))

        # ---- consts on DVE (idle until ~4us; Pool's SWDGE gens must not
        # delay ones16 -> pewarm -> the PE p-state ramp)
        ones16 = const.tile([1, 128], f16, tag="ones16")
        nc.vector.memset(ones16[:], 1.0)
        bias_z = const.tile([128, 1], f32, tag="bz")
        nc.vector.memset(bias_z[:], 0.0)
        dummy = const.tile([1, 128], f16, tag="dummy")
        nc.vector.memset(dummy[:], 0.25)
        bias_hpi = const.tile([128, 1], f32, tag="bhpi")
        nc.vector.memset(bias_hpi[:], PI / 2)
        bias_sh = const.tile([128, 1], f32, tag="bsh")
        nc.vector.memset(bias_sh[:], SHIFT)
        nc.scalar.activation(dummy[:], dummy[:], AF.Sin, bias=bias_z[0:1, :])

        # ---- input DMAs. HWDGE (sync/scalar issue) gens serialize at 625ns
        # on one device; Pool SWDGE is a parallel generator (~1us/dma on the
        # Pool engine). v-projection operands first (v chain is longest).
        vT_sb = work.tile([128, 2 * 512], f16, tag="vT")
        nc.sync.dma_start(vT_sb[:], vT_ap)
        wq_sb = work.tile([128, 2 * 256], f16, tag="wq")
        nc.scalar.dma_start(wq_sb[:], wq_ap)
        fold_sb = const.tile([128, 2 * K + 2], f32, tag="fold")
        nc.sync.dma_start(fold_sb[:], fold_ap)
        ident_sb = const.tile([128, 128], f16, tag="ident")
        nc.scalar.dma_start(ident_sb[:], ident_ap)
        qT_sb = work.tile([128, 2 * 256], f16, tag="qT")
        nc.gpsimd.dma_start(qT_sb[:], qT_ap)
        wv_sb = work.tile([128, 2 * 256], f16, tag="wv")
        nc.gpsimd.dma_start(wv_sb[:], wv_ap)
        cau_sb = const.tile([128, 4 * 256], f16, tag="cau")
        nc.gpsimd.dma_start(cau_sb[:], cau_ap)
        val_sb = work.tile([128, 4 * 257], f16, tag="val")
        nc.gpsimd.dma_start(val_sb[:], val_ap)
        # PSUM: one accumulation group per BANK at a time (start=True
        # invalidates the whole bank). 6 banks: psv0 psv1 (512f32 = 1 bank
        # each), psq0 psq1, ctx0 ctx1. Score regions REUSE the projection
        # banks (projection groups are stopped and fully read by the DVE
        # converts before each score init; the WAR dep rides the AP overlap).
        psq_t = [pp.tile([128, 256], f32, tag=f"psq{ut}", name=f"psq{ut}")
                 for ut in range(2)]

        # ---- projections -> PSUM f32, then DVE converts to f16
        # psv[ut]: [128 u, 512 j], psq[ut]: [128 u, 256 i]
        psv, psq = [], []
        for ut in range(2):
            ps = psq_t[ut]
            if ut == 0:
                # PE p-state warm-up rides a corner of psq0 (wiped by the
                # projection's own start=True; nothing reads it)
                nc.tensor.matmul(
                    ps[:, 0:1], lhsT=ones16[:], rhs=ones16[:, 0:1],
                    start=True, stop=True, skip_group_check=True,
                )
            for dt in range(2):
                nc.tensor.matmul(
                    ps[:],
                    lhsT=wq_sb[:, dt * 256 + ut * 128: dt * 256 + ut * 128 + 128],
                    rhs=qT_sb[:, dt * 256: dt * 256 + 256],
                    start=(dt == 0), stop=(dt == 1),
                )
            psq.append(ps[:])
        for ut in range(2):
            ps = pp.tile([128, 512], f32, tag=f"psv{ut}", name=f"psv{ut}")
            for dt in range(2):
                nc.tensor.matmul(
                    ps[:],
                    lhsT=wv_sb[:, dt * 256 + ut * 128: dt * 256 + ut * 128 + 128],
                    rhs=vT_sb[:, dt * 512: dt * 512 + 512],
                    start=(dt == 0), stop=(dt == 1),
                )
            psv.append(ps)

        projs = {}
        conv_src = {("v", 0): psv[0][:], ("v", 1): psv[1][:],
                    ("q", 0): psq[0], ("q", 1): psq[1]}
        for side, ut, Wd in (("v", 0, 512), ("q", 0, 256),
                             ("v", 1, 512), ("q", 1, 256)):
            projs[(side, ut)] = work.tile(
                [128, Wd], f16, tag=f"pj{side}{ut}", name=f"pj{side}{ut}")

        def emit_conv(side, ut):
            nc.vector.tensor_copy(projs[(side, ut)][:], conv_src[(side, ut)])

        # ---- arg chains (DVE) / features (ACT) / folds (DVE) / scores (PE)
        streams = [("v", 0, 512), ("q", 0, 256), ("v", 1, 512), ("q", 1, 256)]
        r_t, a_t, z_t, u_t, n_t, s_f, c_f = {}, {}, {}, {}, {}, {}, {}
        for side, ut, Wd in streams:
            key = (side, ut)
            r_t[key] = work.tile([128, K * Wd], f16, tag=f"r{side}{ut}", name=f"r{side}{ut}")
            a_t[key] = work.tile([128, K * Wd], f16, tag=f"a{side}{ut}", name=f"a{side}{ut}")
            z_t[key] = work.tile([128, (K - 1) * Wd], f16, tag=f"z{side}{ut}", name=f"z{side}{ut}")
            u_t[key] = work.tile([128, (K - 1) * Wd], f16, tag=f"u{side}{ut}", name=f"u{side}{ut}")
            n_t[key] = work.tile([128, (K - 1) * Wd], f16, tag=f"n{side}{ut}", name=f"n{side}{ut}")
            s_f[key] = work.tile([128, K * Wd], f16, tag=f"s{side}{ut}", name=f"s{side}{ut}")
            c_f[key] = work.tile([128, K * Wd], f16, tag=f"c{side}{ut}", name=f"c{side}{ut}")
        qws = [work.tile([128, K * 256], f16, tag=f"qws{ut}", name=f"qws{ut}") for ut in range(2)]
        qwc = [work.tile([128, K * 256], f16, tag=f"qwc{ut}", name=f"qwc{ut}") for ut in range(2)]

        # chunks of k-space: k0 alone unblocks ACT/PE early; rest split so
        # the tail releases score matmuls finely.
        CHUNKS = [(0, 1), (1, 3), (3, K)]

        def emit_args(side, ut, c0, c1):
            key = (side, ut)
            Wd = 512 if side == "v" else 256
            proj = projs[key]
            r, a, z, u, n = r_t[key], a_t[key], z_t[key], u_t[key], n_t[key]
            for k in range(c0, c1):
                if k == 0:
                    nc.vector.tensor_scalar_mul(
                        r[:, 0:Wd], proj[:], float(-OM[0] / TWO_PI))
                else:
                    nc.vector.tensor_scalar_mul(
                        z[:, (k - 1) * Wd: k * Wd], proj[:],
                        float(OM[k] / TWO_PI))
            z0, z1 = max(c0 - 1, 0), c1 - 1
            if z1 > z0:
                zs = slice(z0 * Wd, z1 * Wd)
                nc.vector.tensor_scalar_add(u[:, zs], z[:, zs], MAGIC)
                nc.vector.tensor_scalar_sub(n[:, zs], u[:, zs], MAGIC)
                nc.vector.tensor_tensor(
                    r[:, (z0 + 1) * Wd: (z1 + 1) * Wd], n[:, zs], z[:, zs],
                    op=OP.subtract)
            nc.vector.tensor_scalar(
                a[:, c0 * Wd: c1 * Wd].bitcast(u16),
                r[:, c0 * Wd: c1 * Wd].bitcast(u16),
                0x7FFF, None, op0=OP.bitwise_and)

        def emit_feats(side, ut, c0, c1):
            key = (side, ut)
            Wd = 512 if side == "v" else 256
            cs = slice(c0 * Wd, c1 * Wd)
            # r ~= n - z  ->  sin(2pi z) = Sin(-2pi r);  cos = Sin(-2pi|r|+pi/2)
            nc.scalar.activation(s_f[key][:, cs], r_t[key][:, cs], AF.Sin,
                                 scale=-TWO_PI, bias=bias_z[:])
            nc.scalar.activation(c_f[key][:, cs], a_t[key][:, cs], AF.Sin,
                                 scale=-TWO_PI, bias=bias_hpi[:])

        def emit_folds(ut, c0, c1):
            for k in range(c0, c1):
                cs = slice(k * 256, (k + 1) * 256)
                col = fold_sb[:, ut * K + k: ut * K + k + 1]
                nc.vector.tensor_scalar(qws[ut][:, cs], s_f[("q", ut)][:, cs],
                                        col, None, op0=OP.mult)
                nc.vector.tensor_scalar(qwc[ut][:, cs], c_f[("q", ut)][:, cs],
                                        col, None, op0=OP.mult)

        # score regions sc[jt]: [128 j, 256 i]; mask-init; extent i0=64*jt
        sc_banks = [psv[0], psv[1], psq_t[0], psq_t[1]]

        def sc_slice(jt, i0, i1):
            return sc_banks[jt][:, i0:i1]

        def emit_init(jt):
            nc.tensor.matmul(
                sc_slice(jt, 0, 256), lhsT=ident_sb[:],
                rhs=cau_sb[:, jt * 256: jt * 256 + 256],
                start=True, stop=False, skip_group_check=True)
        if DEBUG:
            d_init = spool.tile([128, 256], f32, tag="dinit", name="d_init")
            nc.vector.tensor_copy(d_init[:], sc_slice(0, 0, 256))
            nc.sync.dma_start(dbg_aps["d_init0"], d_init[:])
        reg_left = [2 * 2 * K] * 4

        def emit_scores(ut, c0, c1):
            for k in range(c0, c1):
                for jt in range(4):
                    i0 = 64 * jt
                    for lhs_f, rhs_t in ((c_f[("v", ut)], qws[ut]),
                                         (s_f[("v", ut)], qwc[ut])):
                        reg_left[jt] -= 1
                        nc.tensor.matmul(
                            sc_slice(jt, i0, 256),
                            lhsT=lhs_f[:, k * 512 + jt * 128: k * 512 + jt * 128 + 128],
                            rhs=rhs_t[:, k * 256 + i0: k * 256 + 256],
                            start=False, stop=(reg_left[jt] == 0),
                            skip_group_check=True)

        # esc tiles + zero-fill of non-causal columns
        esc = []
        for jt in range(4):
            t = work.tile([128, 256], f16, tag=f"esc{jt}", name=f"esc{jt}")
            if jt > 0:
                nc.gpsimd.memset(t[:, 0: 64 * jt], 0.0)
            esc.append(t)

        # pipeline: ACT opens on q0's features (its projection chain is
        # ~1us shorter); q1 features merged into one (0,K) pair; v stream
        # fills behind. All mask inits precede the first score matmul.
        emit_conv("q", 0)
        emit_args("q", 0, 0, 1)
        emit_feats("q", 0, 0, 1)
        emit_init(2)
        emit_conv("v", 0)
        emit_args("v", 0, 0, 1)
        emit_feats("v", 0, 0, 1)
        emit_init(0)
        emit_conv("q", 1)
        emit_args("q", 1, 0, 1)
        emit_init(3)
        emit_conv("v", 1)
        emit_args("v", 1, 0, 1)
        emit_feats("v", 1, 0, 1)
        emit_init(1)
        emit_args("v", 0, 1, 3)
        emit_feats("v", 0, 1, 3)
        emit_folds(0, 0, 1)
        emit_scores(0, 0, 1)
        emit_args("q", 1, 1, K)
        emit_feats("q", 1, 0, K)
        emit_folds(1, 0, 1)
        emit_scores(1, 0, 1)
        emit_args("q", 0, 1, K)
        emit_feats("q", 0, 1, K)
        emit_args("v", 1, 1, 3)
        emit_feats("v", 1, 1, 3)
        emit_folds(0, 1, 3)
        emit_scores(0, 1, 3)
        emit_folds(1, 1, 3)
        emit_scores(1, 1, 3)
        emit_args("v", 0, 3, K)
        emit_feats("v", 0, 3, K)
        emit_folds(0, 3, K)
        emit_scores(0, 3, K)
        emit_args("v", 1, 3, K)
        emit_feats("v", 1, 3, K)
        emit_folds(1, 3, K)
        emit_scores(1, 3, K)

        # softmax + context
        ctx_ps = []
        for it in range(2):
            t = pp.tile([128, 257], f32, tag=f"ctx{it}", name=f"ctx{it}")
            ctx_ps.append(t)
        for jt in range(4):
            i0 = 64 * jt
            nc.scalar.activation(esc[jt][:, i0:256], sc_slice(jt, i0, 256),
                                 AF.Exp, bias=bias_sh[:])
            for it in range(2):
                nc.tensor.matmul(
                    ctx_ps[it][:],
                    lhsT=esc[jt][:, it * 128: it * 128 + 128],
                    rhs=val_sb[:, jt * 257: jt * 257 + 257],
                    start=(jt == 0), stop=(jt == 3),
                )
        if DEBUG:
            nc.sync.dma_start(dbg_aps["d_pjv0"], projs[("v", 0)][:])
            nc.sync.dma_start(dbg_aps["d_pjq0"], projs[("q", 0)][:])
            nc.sync.dma_start(dbg_aps["d_sfq0"], s_f[("q", 0)][:])
            nc.sync.dma_start(dbg_aps["d_cfv0"], c_f[("v", 0)][:])
            nc.sync.dma_start(dbg_aps["d_qws0"], qws[0][:])
            nc.sync.dma_start(dbg_aps["d_esc0"], esc[0][:])
            nc.sync.dma_start(dbg_aps["d_esc3"], esc[3][:])
            dbg_ctx = spool.tile([128, 257], f32, tag="dbgctx", name="dbgctx")
            nc.vector.tensor_copy(dbg_ctx[:], ctx_ps[0][:])
            nc.sync.dma_start(dbg_aps["d_ctx0"], dbg_ctx[:])
        for it in range(2):
            rcp = spool.tile([128, 1], f32, tag="rcp", name=f"rcp{it}")
            nc.vector.reciprocal(rcp[:], ctx_ps[it][:, 256:257])
            rq = spool.tile([128, 1], f32, tag="rq", name=f"rq{it}")
            nc.vector.tensor_scalar(rq[:], rcp[:],
                                    fold_sb[:, 2 * K + it: 2 * K + it + 1],
                                    None, op0=OP.mult)
            octx = spool.tile([128, 256], f16, tag="octx", name=f"octx{it}")
            nc.vector.tensor_scalar(octx[:], ctx_ps[it][:, 0:256],
                                    rq[:, 0:1], None, op0=OP.mult)
            (nc.sync if it == 0 else nc.scalar).dma_start(
                ctx_ap[it * 128: it * 128 + 128, :], octx[:])

    nc.compile()
    return nc


_NC_CACHE = {}


def _get_nc():
    if "nc" not in _NC_CACHE:
        _NC_CACHE["nc"] = _build_program()
    return _NC_CACHE["nc"]


def _qsel(h):
    return np.arange(h, S, 2)


def build_in_maps(values, mask, Wq, Wv, Vw):
    values = np.asarray(values, dtype=np.float32)
    mask = np.asarray(mask)
    Wq = np.asarray(Wq, dtype=np.float32)
    Wv = np.asarray(Wv, dtype=np.float32)
    Vw = np.asarray(Vw, dtype=np.float32)

    # weights packed dt-major: [128, 2*256], col block dt -> d rows
    wv_p = np.concatenate([Wv[0:128, :], Wv[128:256, :]], axis=1).astype(np.float16)
    wq_p = np.concatenate([Wq[0:128, :], Wq[128:256, :]], axis=1).astype(np.float16)

    jcol = np.arange(S)
    in_maps = []
    for c in range(N_CORES):
        b, h = divmod(c, 2)
        qs = _qsel(h)
        vb = values[b]  # [512, 256]
        vT = vb.T.astype(np.float16)  # [256 d, 512 j]
        vT_p = np.concatenate([vT[0:128, :], vT[128:256, :]], axis=1)
        qT = vb[qs].T.astype(np.float16)  # [256 d, 256 i]
        qT_p = np.concatenate([qT[0:128, :], qT[128:256, :]], axis=1)
        # values+ones packed jt-major: [128, 4*257]
        vo = np.concatenate(
            [vb.astype(np.float16), np.ones((S, 1), np.float16)], axis=1)
        val_p = np.concatenate([vo[128 * jt: 128 * (jt + 1), :]
                                for jt in range(4)], axis=1)
        # causalT[j, i]: invalid iff qs[i] < j or key j masked out
        inval = (qs[None, :] < jcol[:, None]) | (~mask[b])[:, None]
        cauT = (inval * NEG16).astype(np.float16)  # [512 j, 256 i]
        cau_p = np.concatenate([cauT[128 * jt: 128 * (jt + 1), :]
                                for jt in range(4)], axis=1)
        # fold scalars + per-it query mask columns
        fold = np.zeros((128, 2 * K + 2), np.float32)
        for ut in range(2):
            for k in range(K):
                fold[:, ut * K + k] = BK[k] * Vw[128 * ut: 128 * (ut + 1)]
        qm = mask[b][qs].astype(np.float32)
        fold[:, 2 * K] = qm[0:128]
        fold[:, 2 * K + 1] = qm[128:256]
        in_maps.append({
            "wv": wv_p, "wq": wq_p,
            "vT": np.ascontiguousarray(vT_p),
            "qT": np.ascontiguousarray(qT_p),
            "val": np.ascontiguousarray(val_p),
            "cau": np.ascontiguousarray(cau_p),
            "ident": np.eye(128, dtype=np.float16),
            "fold": fold,
        })
    return in_maps


def kernel(values, mask, Wq, Wv, Vw):
    nc = _get_nc()
    in_maps = build_in_maps(values, mask, Wq, Wv, Vw)
    res = run_bass_kernel_spmd(nc, in_maps, list(range(N_CORES)))

    out = np.empty((B, S, D), dtype=np.float32)
    for c in range(N_CORES):
        b, h = divmod(c, 2)
        out[b, _qsel(h)] = res.results[c]["ctx"].astype(np.float32)
    return out


# revision 5
# speedup vs baseline: 1.0467x; 1.0453x over previous
"""Bahdanau additive attention (causal) on 8 TRN2 cores — v2.

Per core (batch b, query-parity h): 256 strided queries i (qs[i]=2i+h),
512 keys j.  score^T layout: regions sc[jt] = [128 keys j, 256 queries i]
in PSUM, computed as sum_u vfeat[u, j-tile] x qfold[u, i] matmuls, so the
exp output esc[jt] = [j, i] is DIRECTLY the ctx matmul's lhsT:
    ctx[i, d] = sum_j esc[j, i] * values[j, d]
-> no attention transposes / copies at all. The values rhs carries a 257th
ones-column so each ctx psum tile's last column accumulates ssum_i for
free. Softmax normalization (and the query mask) is applied per-partition
to the final [i, d] psum tiles.

tanh(x) ~= sum_k b_k sin(nu_k x) (K harmonics, minimax fit on |x|<=8.8,
tail-weighted). Feature args are range-reduced in revolutions via the f16
magic-rounding trick (z, u=z+1536, n=u-1536, r=n-z, a=|r|) on DVE in
4x/2x perf-mode forms; sin/cos features on ACT (Sin activation); the
b_k*Vw_u fold rides per-(ut,k) dual tensor_scalar ops with a per-partition
f32 scalar AP (4x) instead of a tensor-tensor with a materialized fold
tile. Causal+key mask initializes each score region via an identity
matmul; score matmuls cover only the causal column extent [64*jt, 256).
Constant exp shift -4 replaces the row-max pass."""

import sys

sys.path.insert(0, "/opt/trn_rl_repo")

import numpy as np

import concourse.bass as bass
import concourse.bacc as bacc
import concourse.tile as tile
from concourse import mybir
from concourse.bass_utils import run_bass_kernel_spmd

B, S, D, U = 4, 512, 256, 256
N_CORES = 8
NEG16 = -30000.0

f32 = mybir.dt.float32
f16 = mybir.dt.float16
u16 = mybir.dt.uint16
AF = mybir.ActivationFunctionType
OP = mybir.AluOpType

# Minimax fits of tanh on [0, 8.81], tail-weighted (x>6.5 weight 0.25).
# K=5: fit maxerr 8.0e-3, simulated end-to-end rel err 4.4e-3 (HW 4.4e-3).
# K=4: fit maxerr 1.39e-2, simulated end-to-end rel err 1.05e-2.
FITS = {
    5: ([0.300242, 0.906507, 1.525431, 2.171803, 2.850332],
        [1.2297972, 0.3126199, 0.1147746, 0.0450979, 0.0178077]),
    4: ([0.30879, 0.933012, 1.588643, 2.298548],
        [1.2262237, 0.3095406, 0.1132022, 0.0440302]),
}
OM, BK = FITS[5]
K = len(OM)
TWO_PI = 2.0 * np.pi
PI = np.pi
MAGIC = 1536.0
SHIFT = -4.0
DEBUG = False


def _build_program():
    nc = bacc.Bacc("TRN2", target_bir_lowering=False, debug=False)

    # packed inputs (dt/jt-major along free dim so each is ONE dma)
    wv_ap = nc.dram_tensor("wv", [128, 2 * 256], f16, kind="ExternalInput").ap()
    wq_ap = nc.dram_tensor("wq", [128, 2 * 256], f16, kind="ExternalInput").ap()
    vT_ap = nc.dram_tensor("vT", [128, 2 * 512], f16, kind="ExternalInput").ap()
    qT_ap = nc.dram_tensor("qT", [128, 2 * 256], f16, kind="ExternalInput").ap()
    val_ap = nc.dram_tensor("val", [128, 4 * 257], f16, kind="ExternalInput").ap()
    cau_ap = nc.dram_tensor("cau", [128, 4 * 256], f16, kind="ExternalInput").ap()
    ident_ap = nc.dram_tensor("ident", [128, 128], f16, kind="ExternalInput").ap()
    # fold scalars: col ut*K+k = b_k * Vw[128*ut:128*(ut+1)]; col 2K = qmask
    fold_ap = nc.dram_tensor("fold", [128, 2 * K + 2], f32,
                             kind="ExternalInput").ap()
    ctx_ap = nc.dram_tensor("ctx", [256, D], f16, kind="ExternalOutput").ap()
    dbg_aps = {}
    if DEBUG:
        for nm, shape, dt in [("d_pjv0", [128, 512], f16),
                              ("d_pjq0", [128, 256], f16),
                              ("d_sfq0", [128, K * 256], f16),
                              ("d_cfv0", [128, K * 512], f16),
                              ("d_qws0", [128, K * 256], f16),
                              ("d_esc0", [128, 256], f16),
                              ("d_esc3", [128, 256], f16),
                              ("d_ctx0", [128, 257], f32),
                              ("d_init0", [128, 256], f32)]:
            dbg_aps[nm] = nc.dram_tensor(nm, shape, dt, kind="ExternalOutput").ap()

    from contextlib import ExitStack

    with tile.TileContext(nc) as tc, ExitStack() as es:
        const = es.enter_context(tc.tile_pool(name="const", bufs=1))
        work = es.enter_context(tc.tile_pool(name="work", bufs=1))
        spool = es.enter_context(tc.tile_pool(name="small", bufs=4))
        pp = es.enter_context(tc.tile_pool(name="psum", bufs=1, space="PSUM"))

        # ---- consts on DVE (idle until ~4us; Pool's SWDGE gens must not
        # delay ones16 -> pewarm -> the PE p-state ramp)
        ones16 = const.tile([1, 128], f16, tag="ones16")
        nc.vector.memset(ones16[:], 1.0)
        bias_z = const.tile([128, 1], f32, tag="bz")
        nc.vector.memset(bias_z[:], 0.0)
        dummy = const.tile([1, 128], f16, tag="dummy")
        nc.vector.memset(dummy[:], 0.25)
        bias_hpi = const.tile([128, 1], f32, tag="bhpi")
        nc.vector.memset(bias_hpi[:], PI / 2)
        bias_sh = const.tile([128, 1], f32, tag="bsh")
        nc.vector.memset(bias_sh[:], SHIFT)
        nc.scalar.activation(dummy[:], dummy[:], AF.Sin, bias=bias_z[0:1, :])

        # ---- input DMAs. HWDGE (sync/scalar issue) gens serialize at 625ns
        # on one device; Pool SWDGE is a parallel generator (~1us/dma on the
        # Pool engine). v-projection operands first (v chain is longest).
        wq_sb = work.tile([128, 2 * 256], f16, tag="wq")
        nc.sync.dma_start(wq_sb[:], wq_ap)
        vT_sb = work.tile([128, 2 * 512], f16, tag="vT")
        nc.scalar.dma_start(vT_sb[:], vT_ap)
        fold_sb = const.tile([128, 2 * K + 2], f32, tag="fold")
        nc.sync.dma_start(fold_sb[:], fold_ap)
        ident_sb = const.tile([128, 128], f16, tag="ident")
        nc.scalar.dma_start(ident_sb[:], ident_ap)
        qT_sb = work.tile([128, 2 * 256], f16, tag="qT")
        nc.gpsimd.dma_start(qT_sb[:], qT_ap)
        wv_sb = work.tile([128, 2 * 256], f16, tag="wv")
        nc.gpsimd.dma_start(wv_sb[:], wv_ap)
        cau_sb = const.tile([128, 4 * 256], f16, tag="cau")
        nc.gpsimd.dma_start(cau_sb[:], cau_ap)
        val_sb = work.tile([128, 4 * 257], f16, tag="val")
        nc.gpsimd.dma_start(val_sb[:], val_ap)
        # PSUM: one accumulation group per BANK at a time (start=True
        # invalidates the whole bank). 6 banks: psv0 psv1 (512f32 = 1 bank
        # each), psq0 psq1, ctx0 ctx1. Score regions REUSE the projection
        # banks (projection groups are stopped and fully read by the DVE
        # converts before each score init; the WAR dep rides the AP overlap).
        psq_t = [pp.tile([128, 256], f32, tag=f"psq{ut}", name=f"psq{ut}")
                 for ut in range(2)]

        # ---- projections -> PSUM f32, then DVE converts to f16
        # psv[ut]: [128 u, 512 j], psq[ut]: [128 u, 256 i]
        psv, psq = [], []
        for ut in range(2):
            ps = psq_t[ut]
            if ut == 0:
                # PE p-state warm-up rides a corner of psq0 (wiped by the
                # projection's own start=True; nothing reads it)
                nc.tensor.matmul(
                    ps[:, 0:1], lhsT=ones16[:], rhs=ones16[:, 0:1],
                    start=True, stop=True, skip_group_check=True,
                )
            for dt in range(2):
                nc.tensor.matmul(
                    ps[:],
                    lhsT=wq_sb[:, dt * 256 + ut * 128: dt * 256 + ut * 128 + 128],
                    rhs=qT_sb[:, dt * 256: dt * 256 + 256],
                    start=(dt == 0), stop=(dt == 1),
                )
            psq.append(ps[:])
        for ut in range(2):
            ps = pp.tile([128, 512], f32, tag=f"psv{ut}", name=f"psv{ut}")
            for dt in range(2):
                nc.tensor.matmul(
                    ps[:],
                    lhsT=wv_sb[:, dt * 256 + ut * 128: dt * 256 + ut * 128 + 128],
                    rhs=vT_sb[:, dt * 512: dt * 512 + 512],
                    start=(dt == 0), stop=(dt == 1),
                )
            psv.append(ps)

        projs = {}
        conv_src = {("v", 0): psv[0][:], ("v", 1): psv[1][:],
                    ("q", 0): psq[0], ("q", 1): psq[1]}
        for side, ut, Wd in (("v", 0, 512), ("q", 0, 256),
                             ("v", 1, 512), ("q", 1, 256)):
            projs[(side, ut)] = work.tile(
                [128, Wd], f16, tag=f"pj{side}{ut}", name=f"pj{side}{ut}")

        def emit_conv(side, ut):
            nc.vector.tensor_copy(projs[(side, ut)][:], conv_src[(side, ut)])

        # ---- arg chains (DVE) / features (ACT) / folds (DVE) / scores (PE)
        streams = [("v", 0, 512), ("q", 0, 256), ("v", 1, 512), ("q", 1, 256)]
        r_t, a_t, z_t, u_t, n_t, s_f, c_f = {}, {}, {}, {}, {}, {}, {}
        for side, ut, Wd in streams:
            key = (side, ut)
            r_t[key] = work.tile([128, K * Wd], f16, tag=f"r{side}{ut}", name=f"r{side}{ut}")
            a_t[key] = work.tile([128, K * Wd], f16, tag=f"a{side}{ut}", name=f"a{side}{ut}")
            z_t[key] = work.tile([128, (K - 1) * Wd], f16, tag=f"z{side}{ut}", name=f"z{side}{ut}")
            u_t[key] = work.tile([128, (K - 1) * Wd], f16, tag=f"u{side}{ut}", name=f"u{side}{ut}")
            n_t[key] = work.tile([128, (K - 1) * Wd], f16, tag=f"n{side}{ut}", name=f"n{side}{ut}")
            s_f[key] = work.tile([128, K * Wd], f16, tag=f"s{side}{ut}", name=f"s{side}{ut}")
            c_f[key] = work.tile([128, K * Wd], f16, tag=f"c{side}{ut}", name=f"c{side}{ut}")
        qws = [work.tile([128, K * 256], f16, tag=f"qws{ut}", name=f"qws{ut}") for ut in range(2)]
        qwc = [work.tile([128, K * 256], f16, tag=f"qwc{ut}", name=f"qwc{ut}") for ut in range(2)]

        # chunks of k-space: k0 alone unblocks ACT/PE early; rest split so
        # the tail releases score matmuls finely.
        CHUNKS = [(0, 1), (1, 3), (3, K)]

        def emit_args(side, ut, c0, c1):
            key = (side, ut)
            Wd = 512 if side == "v" else 256
            proj = projs[key]
            r, a, z, u, n = r_t[key], a_t[key], z_t[key], u_t[key], n_t[key]
            for k in range(c0, c1):
                if k == 0:
                    nc.vector.tensor_scalar_mul(
                        r[:, 0:Wd], proj[:], float(-OM[0] / TWO_PI))
                else:
                    nc.vector.tensor_scalar_mul(
                        z[:, (k - 1) * Wd: k * Wd], proj[:],
                        float(OM[k] / TWO_PI))
            z0, z1 = max(c0 - 1, 0), c1 - 1
            if z1 > z0:
                zs = slice(z0 * Wd, z1 * Wd)
                nc.vector.tensor_scalar_add(u[:, zs], z[:, zs], MAGIC)
                nc.vector.tensor_scalar_sub(n[:, zs], u[:, zs], MAGIC)
                nc.vector.tensor_tensor(
                    r[:, (z0 + 1) * Wd: (z1 + 1) * Wd], n[:, zs], z[:, zs],
                    op=OP.subtract)
            nc.vector.tensor_scalar(
                a[:, c0 * Wd: c1 * Wd].bitcast(u16),
                r[:, c0 * Wd: c1 * Wd].bitcast(u16),
                0x7FFF, None, op0=OP.bitwise_and)

        def emit_feats(side, ut, c0, c1):
            key = (side, ut)
            Wd = 512 if side == "v" else 256
            cs = slice(c0 * Wd, c1 * Wd)
            # r ~= n - z  ->  sin(2pi z) = Sin(-2pi r);  cos = Sin(-2pi|r|+pi/2)
            nc.scalar.activation(s_f[key][:, cs], r_t[key][:, cs], AF.Sin,
                                 scale=-TWO_PI, bias=bias_z[:])
            nc.scalar.activation(c_f[key][:, cs], a_t[key][:, cs], AF.Sin,
                                 scale=-TWO_PI, bias=bias_hpi[:])

        def emit_folds(ut, c0, c1):
            for k in range(c0, c1):
                cs = slice(k * 256, (k + 1) * 256)
                col = fold_sb[:, ut * K + k: ut * K + k + 1]
                nc.vector.tensor_scalar(qws[ut][:, cs], s_f[("q", ut)][:, cs],
                                        col, None, op0=OP.mult)
                nc.vector.tensor_scalar(qwc[ut][:, cs], c_f[("q", ut)][:, cs],
                                        col, None, op0=OP.mult)

        # score regions sc[jt]: [128 j, 256 i]; mask-init; extent i0=64*jt
        sc_banks = [psv[0], psv[1], psq_t[0], psq_t[1]]

        def sc_slice(jt, i0, i1):
            return sc_banks[jt][:, i0:i1]

        def emit_init(jt):
            nc.tensor.matmul(
                sc_slice(jt, 0, 256), lhsT=ident_sb[:],
                rhs=cau_sb[:, jt * 256: jt * 256 + 256],
                start=True, stop=False, skip_group_check=True)
        if DEBUG:
            d_init = spool.tile([128, 256], f32, tag="dinit", name="d_init")
            nc.vector.tensor_copy(d_init[:], sc_slice(0, 0, 256))
            nc.sync.dma_start(dbg_aps["d_init0"], d_init[:])
        reg_left = [2 * 2 * K] * 4

        def emit_scores(ut, c0, c1):
            for k in range(c0, c1):
                for jt in range(4):
                    i0 = 64 * jt
                    for lhs_f, rhs_t in ((c_f[("v", ut)], qws[ut]),
                                         (s_f[("v", ut)], qwc[ut])):
                        reg_left[jt] -= 1
                        nc.tensor.matmul(
                            sc_slice(jt, i0, 256),
                            lhsT=lhs_f[:, k * 512 + jt * 128: k * 512 + jt * 128 + 128],
                            rhs=rhs_t[:, k * 256 + i0: k * 256 + 256],
                            start=False, stop=(reg_left[jt] == 0),
                            skip_group_check=True)

        # esc tiles + zero-fill of non-causal columns
        esc = []
        for jt in range(4):
            t = work.tile([128, 256], f16, tag=f"esc{jt}", name=f"esc{jt}")
            if jt > 0:
                nc.gpsimd.memset(t[:, 0: 64 * jt], 0.0)
            esc.append(t)

        # pipeline: ACT opens on q0-A, q1-A fills the gap until the v
        # projections land; v stream follows; folds trail their feats.
        emit_conv("q", 0)
        emit_args("q", 0, 0, 1)
        emit_feats("q", 0, 0, 1)
        emit_init(2)
        emit_conv("q", 1)
        emit_args("q", 1, 0, 1)
        emit_feats("q", 1, 0, 1)
        emit_init(3)
        emit_conv("v", 0)
        emit_args("v", 0, 0, 1)
        emit_feats("v", 0, 0, 1)
        emit_init(0)
        emit_conv("v", 1)
        emit_args("v", 1, 0, 1)
        emit_feats("v", 1, 0, 1)
        emit_init(1)
        emit_args("v", 0, 1, 3)
        emit_feats("v", 0, 1, 3)
        emit_folds(0, 0, 1)
        emit_scores(0, 0, 1)
        emit_args("q", 0, 1, K)
        emit_feats("q", 0, 1, K)
        emit_folds(1, 0, 1)
        emit_scores(1, 0, 1)
        emit_args("q", 1, 1, K)
        emit_feats("q", 1, 1, K)
        emit_args("v", 1, 1, 3)
        emit_feats("v", 1, 1, 3)
        emit_folds(0, 1, 3)
        emit_scores(0, 1, 3)
        emit_folds(1, 1, 3)
        emit_scores(1, 1, 3)
        emit_args("v", 0, 3, K)
        emit_feats("v", 0, 3, K)
        emit_folds(0, 3, K)
        emit_scores(0, 3, K)
        emit_args("v", 1, 3, K)
        emit_feats("v", 1, 3, K)
        emit_folds(1, 3, K)
        emit_scores(1, 3, K)

        # softmax + context
        ctx_ps = []
        for it in range(2):
            t = pp.tile([128, 257], f32, tag=f"ctx{it}", name=f"ctx{it}")
            ctx_ps.append(t)
        for jt in range(4):
            i0 = 64 * jt
            nc.scalar.activation(esc[jt][:, i0:256], sc_slice(jt, i0, 256),
                                 AF.Exp, bias=bias_sh[:])
            for it in range(2):
                nc.tensor.matmul(
                    ctx_ps[it][:],
                    lhsT=esc[jt][:, it * 128: it * 128 + 128],
                    rhs=val_sb[:, jt * 257: jt * 257 + 257],
                    start=(jt == 0), stop=(jt == 3),
                )
        if DEBUG:
            nc.sync.dma_start(dbg_aps["d_pjv0"], projs[("v", 0)][:])
            nc.sync.dma_start(dbg_aps["d_pjq0"], projs[("q", 0)][:])
            nc.sync.dma_start(dbg_aps["d_sfq0"], s_f[("q", 0)][:])
            nc.sync.dma_start(dbg_aps["d_cfv0"], c_f[("v", 0)][:])
            nc.sync.dma_start(dbg_aps["d_qws0"], qws[0][:])
            nc.sync.dma_start(dbg_aps["d_esc0"], esc[0][:])
            nc.sync.dma_start(dbg_aps["d_esc3"], esc[3][:])
            dbg_ctx = spool.tile([128, 257], f32, tag="dbgctx", name="dbgctx")
            nc.vector.tensor_copy(dbg_ctx[:], ctx_ps[0][:])
            nc.sync.dma_start(dbg_aps["d_ctx0"], dbg_ctx[:])
        for it in range(2):
            rcp = spool.tile([128, 1], f32, tag="rcp", name=f"rcp{it}")
            nc.vector.reciprocal(rcp[:], ctx_ps[it][:, 256:257])
            rq = spool.tile([128, 1], f32, tag="rq", name=f"rq{it}")
            nc.vector.tensor_scalar(rq[:], rcp[:],
                                    fold_sb[:, 2 * K + it: 2 * K + it + 1],
                                    None, op0=OP.mult)
            octx = spool.tile([128, 256], f16, tag="octx", name=f"octx{it}")
            nc.vector.tensor_scalar(octx[:], ctx_ps[it][:, 0:256],
                                    rq[:, 0:1], None, op0=OP.mult)
            (nc.sync if it == 0 else nc.scalar).dma_start(
                ctx_ap[it * 128: it * 128 + 128, :], octx[:])

    nc.compile()
    return nc


_NC_CACHE = {}


def _get_nc():
    if "nc" not in _NC_CACHE:
        _NC_CACHE["nc"] = _build_program()
    return _NC_CACHE["nc"]


def _qsel(h):
    return np.arange(h, S, 2)


def build_in_maps(values, mask, Wq, Wv, Vw):
    values = np.asarray(values, dtype=np.float32)
    mask = np.asarray(mask)
    Wq = np.asarray(Wq, dtype=np.float32)
    Wv = np.asarray(Wv, dtype=np.float32)
    Vw = np.asarray(Vw, dtype=np.float32)

    # weights packed dt-major: [128, 2*256], col block dt -> d rows
    wv_p = np.concatenate([Wv[0:128, :], Wv[128:256, :]], axis=1).astype(np.float16)
    wq_p = np.concatenate([Wq[0:128, :], Wq[128:256, :]], axis=1).astype(np.float16)

    jcol = np.arange(S)
    in_maps = []
    for c in range(N_CORES):
        b, h = divmod(c, 2)
        qs = _qsel(h)
        vb = values[b]  # [512, 256]
        vT = vb.T.astype(np.float16)  # [256 d, 512 j]
        vT_p = np.concatenate([vT[0:128, :], vT[128:256, :]], axis=1)
        qT = vb[qs].T.astype(np.float16)  # [256 d, 256 i]
        qT_p = np.concatenate([qT[0:128, :], qT[128:256, :]], axis=1)
        # values+ones packed jt-major: [128, 4*257]
        vo = np.concatenate(
            [vb.astype(np.float16), np.ones((S, 1), np.float16)], axis=1)
        val_p = np.concatenate([vo[128 * jt: 128 * (jt + 1), :]
                                for jt in range(4)], axis=1)
        # causalT[j, i]: invalid iff qs[i] < j or key j masked out
        inval = (qs[None, :] < jcol[:, None]) | (~mask[b])[:, None]
        cauT = (inval * NEG16).astype(np.float16)  # [512 j, 256 i]
        cau_p = np.concatenate([cauT[128 * jt: 128 * (jt + 1), :]
                                for jt in range(4)], axis=1)
        # fold scalars + per-it query mask columns
        fold = np.zeros((128, 2 * K + 2), np.float32)
        for ut in range(2):
            for k in range(K):
                fold[:, ut * K + k] = BK[k] * Vw[128 * ut: 128 * (ut + 1)]
        qm = mask[b][qs].astype(np.float32)
        fold[:, 2 * K] = qm[0:128]
        fold[:, 2 * K + 1] = qm[128:256]
        in_maps.append({
            "wv": wv_p, "wq": wq_p,
            "vT": np.ascontiguousarray(vT_p),
            "qT": np.ascontiguousarray(qT_p),
            "val": np.ascontiguousarray(val_p),
            "cau": np.ascontiguousarray(cau_p),
            "ident": np.eye(128, dtype=np.float16),
            "fold": fold,
        })
    return in_maps


def kernel(values, mask, Wq, Wv, Vw):
    nc = _get_nc()
    in_maps = build_in_maps(values, mask, Wq, Wv, Vw)
    res = run_bass_kernel_spmd(nc, in_maps, list(range(N_CORES)))

    out = np.empty((B, S, D), dtype=np.float32)
    for c in range(N_CORES):
        b, h = divmod(c, 2)
        out[b, _qsel(h)] = res.results[c]["ctx"].astype(np.float32)
    return out


# revision 6
# speedup vs baseline: 1.1756x; 1.1232x over previous
"""Bahdanau additive attention (causal) on 8 TRN2 cores — v2.

Per core (batch b, query-parity h): 256 strided queries i (qs[i]=2i+h),
512 keys j.  score^T layout: regions sc[jt] = [128 keys j, 256 queries i]
in PSUM, computed as sum_u vfeat[u, j-tile] x qfold[u, i] matmuls, so the
exp output esc[jt] = [j, i] is DIRECTLY the ctx matmul's lhsT:
    ctx[i, d] = sum_j esc[j, i] * values[j, d]
-> no attention transposes / copies at all. The values rhs carries a 257th
ones-column so each ctx psum tile's last column accumulates ssum_i for
free. Softmax normalization (and the query mask) is applied per-partition
to the final [i, d] psum tiles.

tanh(x) ~= sum_k b_k sin(nu_k x) (K harmonics, minimax fit on |x|<=8.8,
tail-weighted). Feature args are range-reduced in revolutions via the f16
magic-rounding trick (z, u=z+1536, n=u-1536, r=n-z, a=|r|) on DVE in
4x/2x perf-mode forms; sin/cos features on ACT (Sin activation); the
b_k*Vw_u fold rides per-(ut,k) dual tensor_scalar ops with a per-partition
f32 scalar AP (4x) instead of a tensor-tensor with a materialized fold
tile. Causal+key mask initializes each score region via an identity
matmul; score matmuls cover only the causal column extent [64*jt, 256).
Constant exp shift -4 replaces the row-max pass."""

import sys

sys.path.insert(0, "/opt/trn_rl_repo")

import numpy as np

import concourse.bass as bass
import concourse.bacc as bacc
import concourse.tile as tile
from concourse import mybir
from concourse.bass_utils import run_bass_kernel_spmd

B, S, D, U = 4, 512, 256, 256
N_CORES = 8
NEG16 = -30000.0

f32 = mybir.dt.float32
f16 = mybir.dt.float16
u16 = mybir.dt.uint16
AF = mybir.ActivationFunctionType
OP = mybir.AluOpType

# Minimax fits of tanh on [0, 8.81], tail-weighted (x>6.5 weight 0.25).
# K=5: fit maxerr 8.0e-3, simulated end-to-end rel err 4.4e-3 (HW 4.4e-3).
# K=4: fit maxerr 1.39e-2, simulated end-to-end rel err 1.05e-2.
FITS = {
    5: ([0.300242, 0.906507, 1.525431, 2.171803, 2.850332],
        [1.2297972, 0.3126199, 0.1147746, 0.0450979, 0.0178077]),
    4: ([0.30879, 0.933012, 1.588643, 2.298548],
        [1.2262237, 0.3095406, 0.1132022, 0.0440302]),
}
OM, BK = FITS[5]
K = len(OM)
TWO_PI = 2.0 * np.pi
PI = np.pi
MAGIC = 1536.0
SHIFT = -4.0
DEBUG = False


def _build_program():
    nc = bacc.Bacc("TRN2", target_bir_lowering=False, debug=False)

    # packed inputs (dt/jt-major along free dim so each is ONE dma)
    wv_ap = nc.dram_tensor("wv", [128, 2 * 256], f16, kind="ExternalInput").ap()
    wq_ap = nc.dram_tensor("wq", [128, 2 * 256], f16, kind="ExternalInput").ap()
    vT_ap = nc.dram_tensor("vT", [128, 2 * 512], f16, kind="ExternalInput").ap()
    qT_ap = nc.dram_tensor("qT", [128, 2 * 256], f16, kind="ExternalInput").ap()
    val_ap = nc.dram_tensor("val", [128, 4 * 257], f16, kind="ExternalInput").ap()
    cau_ap = nc.dram_tensor("cau", [128, 4 * 256], f16, kind="ExternalInput").ap()
    ident_ap = nc.dram_tensor("ident", [128, 128], f16, kind="ExternalInput").ap()
    # fold scalars: col ut*K+k = b_k * Vw[128*ut:128*(ut+1)]; col 2K = qmask
    fold_ap = nc.dram_tensor("fold", [128, 2 * K + 2], f32,
                             kind="ExternalInput").ap()
    ctx_ap = nc.dram_tensor("ctx", [256, D], f16, kind="ExternalOutput").ap()
    dbg_aps = {}
    if DEBUG:
        for nm, shape, dt in [("d_pjv0", [128, 512], f16),
                              ("d_pjq0", [128, 256], f16),
                              ("d_sfq0", [128, K * 256], f16),
                              ("d_cfv0", [128, K * 512], f16),
                              ("d_qws0", [128, K * 256], f16),
                              ("d_esc0", [128, 256], f16),
                              ("d_esc3", [128, 256], f16),
                              ("d_ctx0", [128, 257], f32),
                              ("d_init0", [128, 256], f32)]:
            dbg_aps[nm] = nc.dram_tensor(nm, shape, dt, kind="ExternalOutput").ap()

    from contextlib import ExitStack

    with tile.TileContext(nc) as tc, ExitStack() as es:
        const = es.enter_context(tc.tile_pool(name="const", bufs=1))
        work = es.enter_context(tc.tile_pool(name="work", bufs=1))
        spool = es.enter_context(tc.tile_pool(name="small", bufs=4))
        pp = es.enter_context(tc.tile_pool(name="psum", bufs=1, space="PSUM"))

        # ---- consts on DVE (idle until ~4us; Pool's SWDGE gens must not
        # delay ones16 -> pewarm -> the PE p-state ramp)
        ones16 = const.tile([1, 128], f16, tag="ones16")
        nc.vector.memset(ones16[:], 1.0)
        bias_z = const.tile([128, 1], f32, tag="bz")
        nc.vector.memset(bias_z[:], 0.0)
        dummy = const.tile([1, 128], f16, tag="dummy")
        nc.vector.memset(dummy[:], 0.25)
        bias_hpi = const.tile([128, 1], f32, tag="bhpi")
        nc.vector.memset(bias_hpi[:], PI / 2)
        bias_sh = const.tile([128, 1], f32, tag="bsh")
        nc.vector.memset(bias_sh[:], SHIFT)
        nc.scalar.activation(dummy[:], dummy[:], AF.Sin, bias=bias_z[0:1, :])

        # ---- input DMAs. HWDGE (sync/scalar issue) gens serialize at 625ns
        # on one device; Pool SWDGE is a parallel generator (~1us/dma on the
        # Pool engine). v-projection operands first (v chain is longest).
        wq_sb = work.tile([128, 2 * 256], f16, tag="wq")
        nc.sync.dma_start(wq_sb[:], wq_ap)
        vT_sb = work.tile([128, 2 * 512], f16, tag="vT")
        nc.scalar.dma_start(vT_sb[:], vT_ap)
        fold_sb = const.tile([128, 2 * K + 2], f32, tag="fold")
        nc.sync.dma_start(fold_sb[:], fold_ap)
        ident_sb = const.tile([128, 128], f16, tag="ident")
        nc.scalar.dma_start(ident_sb[:], ident_ap)
        qT_sb = work.tile([128, 2 * 256], f16, tag="qT")
        nc.gpsimd.dma_start(qT_sb[:], qT_ap)
        wv_sb = work.tile([128, 2 * 256], f16, tag="wv")
        nc.gpsimd.dma_start(wv_sb[:], wv_ap)
        cau_sb = const.tile([128, 4 * 256], f16, tag="cau")
        nc.gpsimd.dma_start(cau_sb[:], cau_ap)
        val_sb = work.tile([128, 4 * 257], f16, tag="val")
        nc.gpsimd.dma_start(val_sb[:], val_ap)
        # PSUM: one accumulation group per BANK at a time (start=True
        # invalidates the whole bank). 6 banks: psv0 psv1 (512f32 = 1 bank
        # each), psq0 psq1, ctx0 ctx1. Score regions REUSE the projection
        # banks (projection groups are stopped and fully read by the DVE
        # converts before each score init; the WAR dep rides the AP overlap).
        psq_t = [pp.tile([128, 256], f32, tag=f"psq{ut}", name=f"psq{ut}")
                 for ut in range(2)]

        # ---- projections -> PSUM f32, then DVE converts to f16
        # psv[ut]: [128 u, 512 j], psq[ut]: [128 u, 256 i]
        psv, psq = [], []
        for ut in range(2):
            ps = psq_t[ut]
            if ut == 0:
                # PE p-state warm-up rides a corner of psq0 (wiped by the
                # projection's own start=True; nothing reads it)
                nc.tensor.matmul(
                    ps[:, 0:1], lhsT=ones16[:], rhs=ones16[:, 0:1],
                    start=True, stop=True, skip_group_check=True,
                )
            for dt in range(2):
                nc.tensor.matmul(
                    ps[:],
                    lhsT=wq_sb[:, dt * 256 + ut * 128: dt * 256 + ut * 128 + 128],
                    rhs=qT_sb[:, dt * 256: dt * 256 + 256],
                    start=(dt == 0), stop=(dt == 1),
                )
            psq.append(ps[:])
        for ut in range(2):
            ps = pp.tile([128, 512], f32, tag=f"psv{ut}", name=f"psv{ut}")
            for dt in range(2):
                nc.tensor.matmul(
                    ps[:],
                    lhsT=wv_sb[:, dt * 256 + ut * 128: dt * 256 + ut * 128 + 128],
                    rhs=vT_sb[:, dt * 512: dt * 512 + 512],
                    start=(dt == 0), stop=(dt == 1),
                )
            psv.append(ps)

        projs = {}
        conv_src = {("v", 0): psv[0][:], ("v", 1): psv[1][:],
                    ("q", 0): psq[0], ("q", 1): psq[1]}
        for side, ut, Wd in (("v", 0, 512), ("q", 0, 256),
                             ("v", 1, 512), ("q", 1, 256)):
            projs[(side, ut)] = work.tile(
                [128, Wd], f16, tag=f"pj{side}{ut}", name=f"pj{side}{ut}")

        def emit_conv(side, ut):
            # q0's copy rides ACT's idle head (ACT's first sin is later);
            # the rest stay on DVE
            if (side, ut) == ("q", 0):
                nc.scalar.copy(projs[(side, ut)][:], conv_src[(side, ut)])
            else:
                nc.vector.tensor_copy(projs[(side, ut)][:], conv_src[(side, ut)])

        # ---- arg chains (DVE) / features (ACT) / folds (DVE) / scores (PE)
        streams = [("v", 0, 512), ("q", 0, 256), ("v", 1, 512), ("q", 1, 256)]
        r_t, a_t, z_t, u_t, n_t, s_f, c_f = {}, {}, {}, {}, {}, {}, {}
        for side, ut, Wd in streams:
            key = (side, ut)
            r_t[key] = work.tile([128, K * Wd], f16, tag=f"r{side}{ut}", name=f"r{side}{ut}")
            a_t[key] = work.tile([128, K * Wd], f16, tag=f"a{side}{ut}", name=f"a{side}{ut}")
            z_t[key] = work.tile([128, (K - 1) * Wd], f16, tag=f"z{side}{ut}", name=f"z{side}{ut}")
            u_t[key] = work.tile([128, (K - 1) * Wd], f16, tag=f"u{side}{ut}", name=f"u{side}{ut}")
            n_t[key] = work.tile([128, (K - 1) * Wd], f16, tag=f"n{side}{ut}", name=f"n{side}{ut}")
            s_f[key] = work.tile([128, K * Wd], f16, tag=f"s{side}{ut}", name=f"s{side}{ut}")
            c_f[key] = work.tile([128, K * Wd], f16, tag=f"c{side}{ut}", name=f"c{side}{ut}")
        qws = [work.tile([128, K * 256], f16, tag=f"qws{ut}", name=f"qws{ut}") for ut in range(2)]
        qwc = [work.tile([128, K * 256], f16, tag=f"qwc{ut}", name=f"qwc{ut}") for ut in range(2)]

        # chunks of k-space: k0 alone unblocks ACT/PE early; rest split so
        # the tail releases score matmuls finely.
        CHUNKS = [(0, 1), (1, 3), (3, K)]

        def emit_args(side, ut, c0, c1):
            key = (side, ut)
            Wd = 512 if side == "v" else 256
            proj = projs[key]
            r, a, z, u, n = r_t[key], a_t[key], z_t[key], u_t[key], n_t[key]
            for k in range(c0, c1):
                if k == 0:
                    nc.vector.tensor_scalar_mul(
                        r[:, 0:Wd], proj[:], float(-OM[0] / TWO_PI))
                else:
                    nc.vector.tensor_scalar_mul(
                        z[:, (k - 1) * Wd: k * Wd], proj[:],
                        float(OM[k] / TWO_PI))
            z0, z1 = max(c0 - 1, 0), c1 - 1
            if z1 > z0:
                zs = slice(z0 * Wd, z1 * Wd)
                nc.vector.tensor_scalar_add(u[:, zs], z[:, zs], MAGIC)
                nc.vector.tensor_scalar_sub(n[:, zs], u[:, zs], MAGIC)
                nc.vector.tensor_tensor(
                    r[:, (z0 + 1) * Wd: (z1 + 1) * Wd], n[:, zs], z[:, zs],
                    op=OP.subtract)
            nc.vector.tensor_scalar(
                a[:, c0 * Wd: c1 * Wd].bitcast(u16),
                r[:, c0 * Wd: c1 * Wd].bitcast(u16),
                0x7FFF, None, op0=OP.bitwise_and)

        def emit_feats(side, ut, c0, c1):
            key = (side, ut)
            Wd = 512 if side == "v" else 256
            cs = slice(c0 * Wd, c1 * Wd)
            # r ~= n - z  ->  sin(2pi z) = Sin(-2pi r);  cos = Sin(-2pi|r|+pi/2)
            nc.scalar.activation(s_f[key][:, cs], r_t[key][:, cs], AF.Sin,
                                 scale=-TWO_PI, bias=bias_z[:])
            nc.scalar.activation(c_f[key][:, cs], a_t[key][:, cs], AF.Sin,
                                 scale=-TWO_PI, bias=bias_hpi[:])

        def emit_folds(ut, c0, c1):
            for k in range(c0, c1):
                cs = slice(k * 256, (k + 1) * 256)
                col = fold_sb[:, ut * K + k: ut * K + k + 1]
                nc.vector.tensor_scalar(qws[ut][:, cs], s_f[("q", ut)][:, cs],
                                        col, None, op0=OP.mult)
                nc.vector.tensor_scalar(qwc[ut][:, cs], c_f[("q", ut)][:, cs],
                                        col, None, op0=OP.mult)

        # score regions sc[jt]: [128 j, 256 i]; mask-init; extent i0=64*jt
        sc_banks = [psv[0], psv[1], psq_t[0], psq_t[1]]

        def sc_slice(jt, i0, i1):
            return sc_banks[jt][:, i0:i1]

        def emit_init(jt):
            nc.tensor.matmul(
                sc_slice(jt, 0, 256), lhsT=ident_sb[:],
                rhs=cau_sb[:, jt * 256: jt * 256 + 256],
                start=True, stop=False, skip_group_check=True)
        if DEBUG:
            d_init = spool.tile([128, 256], f32, tag="dinit", name="d_init")
            nc.vector.tensor_copy(d_init[:], sc_slice(0, 0, 256))
            nc.sync.dma_start(dbg_aps["d_init0"], d_init[:])
        reg_left = [2 * 2 * K] * 4

        def emit_scores(ut, c0, c1):
            for k in range(c0, c1):
                for jt in range(4):
                    i0 = 64 * jt
                    for lhs_f, rhs_t in ((c_f[("v", ut)], qws[ut]),
                                         (s_f[("v", ut)], qwc[ut])):
                        reg_left[jt] -= 1
                        nc.tensor.matmul(
                            sc_slice(jt, i0, 256),
                            lhsT=lhs_f[:, k * 512 + jt * 128: k * 512 + jt * 128 + 128],
                            rhs=rhs_t[:, k * 256 + i0: k * 256 + 256],
                            start=False, stop=(reg_left[jt] == 0),
                            skip_group_check=True)

        # esc tiles + zero-fill of non-causal columns
        esc = []
        for jt in range(4):
            t = work.tile([128, 256], f16, tag=f"esc{jt}", name=f"esc{jt}")
            if jt > 0:
                nc.gpsimd.memset(t[:, 0: 64 * jt], 0.0)
            esc.append(t)

        # pipeline: ACT opens on q0-A, q1-A fills the gap until the v
        # projections land; v stream follows; folds trail their feats.
        emit_conv("q", 0)
        emit_args("q", 0, 0, 1)
        emit_feats("q", 0, 0, 1)
        emit_init(2)
        emit_conv("q", 1)
        emit_args("q", 1, 0, 1)
        emit_feats("q", 1, 0, 1)
        emit_init(3)
        emit_conv("v", 0)
        emit_args("v", 0, 0, 1)
        emit_feats("v", 0, 0, 1)
        emit_init(0)
        emit_conv("v", 1)
        emit_args("v", 1, 0, 1)
        emit_feats("v", 1, 0, 1)
        emit_init(1)
        emit_args("v", 0, 1, 3)
        emit_feats("v", 0, 1, 3)
        emit_folds(0, 0, 1)
        emit_scores(0, 0, 1)
        emit_args("q", 0, 1, K)
        emit_feats("q", 0, 1, K)
        emit_folds(1, 0, 1)
        emit_scores(1, 0, 1)
        emit_args("q", 1, 1, K)
        emit_feats("q", 1, 1, K)
        emit_args("v", 1, 1, 3)
        emit_feats("v", 1, 1, 3)
        emit_args("v", 0, 3, K)
        emit_feats("v", 0, 3, K)
        emit_folds(0, 1, 3)
        emit_scores(0, 1, 3)
        emit_folds(1, 1, 3)
        emit_scores(1, 1, 3)
        emit_args("v", 1, 3, K)
        emit_feats("v", 1, 3, K)
        emit_folds(0, 3, K)
        emit_scores(0, 3, K)
        emit_folds(1, 3, K)
        emit_scores(1, 3, K)

        # softmax + context
        ctx_ps = []
        for it in range(2):
            t = pp.tile([128, 257], f32, tag=f"ctx{it}", name=f"ctx{it}")
            ctx_ps.append(t)
        for jt in range(4):
            i0 = 64 * jt
            nc.scalar.activation(esc[jt][:, i0:256], sc_slice(jt, i0, 256),
                                 AF.Exp, bias=bias_sh[:])
            for it in range(2):
                nc.tensor.matmul(
                    ctx_ps[it][:],
                    lhsT=esc[jt][:, it * 128: it * 128 + 128],
                    rhs=val_sb[:, jt * 257: jt * 257 + 257],
                    start=(jt == 0), stop=(jt == 3),
                )
        if DEBUG:
            nc.sync.dma_start(dbg_aps["d_pjv0"], projs[("v", 0)][:])
            nc.sync.dma_start(dbg_aps["d_pjq0"], projs[("q", 0)][:])
            nc.sync.dma_start(dbg_aps["d_sfq0"], s_f[("q", 0)][:])
            nc.sync.dma_start(dbg_aps["d_cfv0"], c_f[("v", 0)][:])
            nc.sync.dma_start(dbg_aps["d_qws0"], qws[0][:])
            nc.sync.dma_start(dbg_aps["d_esc0"], esc[0][:])
            nc.sync.dma_start(dbg_aps["d_esc3"], esc[3][:])
            dbg_ctx = spool.tile([128, 257], f32, tag="dbgctx", name="dbgctx")
            nc.vector.tensor_copy(dbg_ctx[:], ctx_ps[0][:])
            nc.sync.dma_start(dbg_aps["d_ctx0"], dbg_ctx[:])
        for it in range(2):
            rcp = spool.tile([128, 1], f32, tag="rcp", name=f"rcp{it}")
            nc.vector.reciprocal(rcp[:], ctx_ps[it][:, 256:257])
            rq = spool.tile([128, 1], f32, tag="rq", name=f"rq{it}")
            nc.vector.tensor_scalar(rq[:], rcp[:],
                                    fold_sb[:, 2 * K + it: 2 * K + it + 1],
                                    None, op0=OP.mult)
            octx = spool.tile([128, 256], f16, tag="octx", name=f"octx{it}")
            nc.vector.tensor_scalar(octx[:], ctx_ps[it][:, 0:256],
                                    rq[:, 0:1], None, op0=OP.mult)
            (nc.sync if it == 0 else nc.scalar).dma_start(
                ctx_ap[it * 128: it * 128 + 128, :], octx[:])

    nc.compile()
    return nc


_NC_CACHE = {}


def _get_nc():
    if "nc" not in _NC_CACHE:
        _NC_CACHE["nc"] = _build_program()
    return _NC_CACHE["nc"]


def _qsel(h):
    return np.arange(h, S, 2)


def build_in_maps(values, mask, Wq, Wv, Vw):
    values = np.asarray(values, dtype=np.float32)
    mask = np.asarray(mask)
    Wq = np.asarray(Wq, dtype=np.float32)
    Wv = np.asarray(Wv, dtype=np.float32)
    Vw = np.asarray(Vw, dtype=np.float32)

    # weights packed dt-major: [128, 2*256], col block dt -> d rows
    wv_p = np.concatenate([Wv[0:128, :], Wv[128:256, :]], axis=1).astype(np.float16)
    wq_p = np.concatenate([Wq[0:128, :], Wq[128:256, :]], axis=1).astype(np.float16)

    jcol = np.arange(S)
    in_maps = []
    for c in range(N_CORES):
        b, h = divmod(c, 2)
        qs = _qsel(h)
        vb = values[b]  # [512, 256]
        vT = vb.T.astype(np.float16)  # [256 d, 512 j]
        vT_p = np.concatenate([vT[0:128, :], vT[128:256, :]], axis=1)
        qT = vb[qs].T.astype(np.float16)  # [256 d, 256 i]
        qT_p = np.concatenate([qT[0:128, :], qT[128:256, :]], axis=1)
        # values+ones packed jt-major: [128, 4*257]
        vo = np.concatenate(
            [vb.astype(np.float16), np.ones((S, 1), np.float16)], axis=1)
        val_p = np.concatenate([vo[128 * jt: 128 * (jt + 1), :]
                                for jt in range(4)], axis=1)
        # causalT[j, i]: invalid iff qs[i] < j or key j masked out
        inval = (qs[None, :] < jcol[:, None]) | (~mask[b])[:, None]
        cauT = (inval * NEG16).astype(np.float16)  # [512 j, 256 i]
        cau_p = np.concatenate([cauT[128 * jt: 128 * (jt + 1), :]
                                for jt in range(4)], axis=1)
        # fold scalars + per-it query mask columns
        fold = np.zeros((128, 2 * K + 2), np.float32)
        for ut in range(2):
            for k in range(K):
                fold[:, ut * K + k] = BK[k] * Vw[128 * ut: 128 * (ut + 1)]
        qm = mask[b][qs].astype(np.float32)
        fold[:, 2 * K] = qm[0:128]
        fold[:, 2 * K + 1] = qm[128:256]
        in_maps.append({
            "wv": wv_p, "wq": wq_p,
            "vT": np.ascontiguousarray(vT_p),
            "qT": np.ascontiguousarray(qT_p),
            "val": np.ascontiguousarray(val_p),
            "cau": np.ascontiguousarray(cau_p),
            "ident": np.eye(128, dtype=np.float16),
            "fold": fold,
        })
    return in_maps


def kernel(values, mask, Wq, Wv, Vw):
    nc = _get_nc()
    in_maps = build_in_maps(values, mask, Wq, Wv, Vw)
    res = run_bass_kernel_spmd(nc, in_maps, list(range(N_CORES)))

    out = np.empty((B, S, D), dtype=np.float32)
    for c in range(N_CORES):
        b, h = divmod(c, 2)
        out[b, _qsel(h)] = res.results[c]["ctx"].astype(np.float32)
    return out


# revision 7
# speedup vs baseline: 1.3633x; 1.1597x over previous
"""Bahdanau additive attention (causal) on 8 TRN2 cores — v2.

Per core (batch b, query-parity h): 256 strided queries i (qs[i]=2i+h),
512 keys j.  score^T layout: regions sc[jt] = [128 keys j, 256 queries i]
in PSUM, computed as sum_u vfeat[u, j-tile] x qfold[u, i] matmuls, so the
exp output esc[jt] = [j, i] is DIRECTLY the ctx matmul's lhsT:
    ctx[i, d] = sum_j esc[j, i] * values[j, d]
-> no attention transposes / copies at all. The values rhs carries a 257th
ones-column so each ctx psum tile's last column accumulates ssum_i for
free. Softmax normalization (and the query mask) is applied per-partition
to the final [i, d] psum tiles.

tanh(x) ~= sum_k b_k sin(nu_k x) (K harmonics, minimax fit on |x|<=8.8,
tail-weighted). Feature args are range-reduced in revolutions via the f16
magic-rounding trick (z, u=z+1536, n=u-1536, r=n-z, a=|r|) on DVE in
4x/2x perf-mode forms; sin/cos features on ACT (Sin activation); the
b_k*Vw_u fold rides per-(ut,k) dual tensor_scalar ops with a per-partition
f32 scalar AP (4x) instead of a tensor-tensor with a materialized fold
tile. Causal+key mask initializes each score region via an identity
matmul; score matmuls cover only the causal column extent [64*jt, 256).
Constant exp shift -4 replaces the row-max pass."""

import sys

sys.path.insert(0, "/opt/trn_rl_repo")

import numpy as np

import concourse.bass as bass
import concourse.bacc as bacc
import concourse.tile as tile
from concourse import mybir
from concourse.bass_utils import run_bass_kernel_spmd

B, S, D, U = 4, 512, 256, 256
N_CORES = 8
NEG16 = -30000.0

f32 = mybir.dt.float32
f16 = mybir.dt.float16
u16 = mybir.dt.uint16
AF = mybir.ActivationFunctionType
OP = mybir.AluOpType

# Minimax fits of tanh on [0, 8.81], tail-weighted (x>6.5 weight 0.25).
# K=5: fit maxerr 8.0e-3, simulated end-to-end rel err 4.4e-3 (HW 4.4e-3).
# K=4: fit maxerr 1.39e-2, simulated end-to-end rel err 1.05e-2.
FITS = {
    5: ([0.300242, 0.906507, 1.525431, 2.171803, 2.850332],
        [1.2297972, 0.3126199, 0.1147746, 0.0450979, 0.0178077]),
    4: ([0.30879, 0.933012, 1.588643, 2.298548],
        [1.2262237, 0.3095406, 0.1132022, 0.0440302]),
}
OM, BK = FITS[4]
K = len(OM)
TWO_PI = 2.0 * np.pi
PI = np.pi
MAGIC = 1536.0
SHIFT = -4.0
DEBUG = False


def _build_program():
    nc = bacc.Bacc("TRN2", target_bir_lowering=False, debug=False)

    # packed inputs (dt/jt-major along free dim so each is ONE dma)
    wv_ap = nc.dram_tensor("wv", [128, 2 * 256], f16, kind="ExternalInput").ap()
    wq_ap = nc.dram_tensor("wq", [128, 2 * 256], f16, kind="ExternalInput").ap()
    vT_ap = nc.dram_tensor("vT", [128, 2 * 512], f16, kind="ExternalInput").ap()
    qT_ap = nc.dram_tensor("qT", [128, 2 * 256], f16, kind="ExternalInput").ap()
    val_ap = nc.dram_tensor("val", [128, 4 * 257], f16, kind="ExternalInput").ap()
    cau_ap = nc.dram_tensor("cau", [128, 4 * 256], f16, kind="ExternalInput").ap()
    ident_ap = nc.dram_tensor("ident", [128, 128], f16, kind="ExternalInput").ap()
    # fold scalars: col ut*K+k = b_k * Vw[128*ut:128*(ut+1)]; col 2K = qmask
    fold_ap = nc.dram_tensor("fold", [128, 2 * K + 2], f32,
                             kind="ExternalInput").ap()
    ctx_ap = nc.dram_tensor("ctx", [256, D], f16, kind="ExternalOutput").ap()
    dbg_aps = {}
    if DEBUG:
        for nm, shape, dt in [("d_pjv0", [128, 512], f16),
                              ("d_pjq0", [128, 256], f16),
                              ("d_sfq0", [128, K * 256], f16),
                              ("d_cfv0", [128, K * 512], f16),
                              ("d_qws0", [128, K * 256], f16),
                              ("d_esc0", [128, 256], f16),
                              ("d_esc3", [128, 256], f16),
                              ("d_ctx0", [128, 257], f32),
                              ("d_init0", [128, 256], f32)]:
            dbg_aps[nm] = nc.dram_tensor(nm, shape, dt, kind="ExternalOutput").ap()

    from contextlib import ExitStack

    with tile.TileContext(nc) as tc, ExitStack() as es:
        const = es.enter_context(tc.tile_pool(name="const", bufs=1))
        work = es.enter_context(tc.tile_pool(name="work", bufs=1))
        spool = es.enter_context(tc.tile_pool(name="small", bufs=4))
        pp = es.enter_context(tc.tile_pool(name="psum", bufs=1, space="PSUM"))

        # ---- consts on DVE (idle until ~4us; Pool's SWDGE gens must not
        # delay ones16 -> pewarm -> the PE p-state ramp)
        ones16 = const.tile([1, 128], f16, tag="ones16")
        nc.vector.memset(ones16[:], 1.0)
        bias_z = const.tile([128, 1], f32, tag="bz")
        nc.vector.memset(bias_z[:], 0.0)
        dummy = const.tile([1, 128], f16, tag="dummy")
        nc.vector.memset(dummy[:], 0.25)
        bias_hpi = const.tile([128, 1], f32, tag="bhpi")
        nc.vector.memset(bias_hpi[:], PI / 2)
        bias_sh = const.tile([128, 1], f32, tag="bsh")
        nc.vector.memset(bias_sh[:], SHIFT)
        nc.scalar.activation(dummy[:], dummy[:], AF.Sin, bias=bias_z[0:1, :])

        # ---- input DMAs. HWDGE (sync/scalar issue) gens serialize at 625ns
        # on one device; Pool SWDGE is a parallel generator (~1us/dma on the
        # Pool engine). v-projection operands first (v chain is longest).
        wq_sb = work.tile([128, 2 * 256], f16, tag="wq")
        nc.sync.dma_start(wq_sb[:], wq_ap)
        vT_sb = work.tile([128, 2 * 512], f16, tag="vT")
        nc.scalar.dma_start(vT_sb[:], vT_ap)
        fold_sb = const.tile([128, 2 * K + 2], f32, tag="fold")
        nc.sync.dma_start(fold_sb[:], fold_ap)
        ident_sb = const.tile([128, 128], f16, tag="ident")
        nc.scalar.dma_start(ident_sb[:], ident_ap)
        qT_sb = work.tile([128, 2 * 256], f16, tag="qT")
        nc.gpsimd.dma_start(qT_sb[:], qT_ap)
        wv_sb = work.tile([128, 2 * 256], f16, tag="wv")
        nc.gpsimd.dma_start(wv_sb[:], wv_ap)
        cau_sb = const.tile([128, 4 * 256], f16, tag="cau")
        nc.gpsimd.dma_start(cau_sb[:], cau_ap)
        val_sb = work.tile([128, 4 * 257], f16, tag="val")
        nc.gpsimd.dma_start(val_sb[:], val_ap)
        # PSUM: one accumulation group per BANK at a time (start=True
        # invalidates the whole bank). 6 banks: psv0 psv1 (512f32 = 1 bank
        # each), psq0 psq1, ctx0 ctx1. Score regions REUSE the projection
        # banks (projection groups are stopped and fully read by the DVE
        # converts before each score init; the WAR dep rides the AP overlap).
        psq_t = [pp.tile([128, 256], f32, tag=f"psq{ut}", name=f"psq{ut}")
                 for ut in range(2)]

        # ---- projections -> PSUM f32, then DVE converts to f16
        # psv[ut]: [128 u, 512 j], psq[ut]: [128 u, 256 i]
        psv, psq = [], []
        for ut in range(2):
            ps = psq_t[ut]
            if ut == 0:
                # PE p-state warm-up rides a corner of psq0 (wiped by the
                # projection's own start=True; nothing reads it)
                nc.tensor.matmul(
                    ps[:, 0:1], lhsT=ones16[:], rhs=ones16[:, 0:1],
                    start=True, stop=True, skip_group_check=True,
                )
            for dt in range(2):
                nc.tensor.matmul(
                    ps[:],
                    lhsT=wq_sb[:, dt * 256 + ut * 128: dt * 256 + ut * 128 + 128],
                    rhs=qT_sb[:, dt * 256: dt * 256 + 256],
                    start=(dt == 0), stop=(dt == 1),
                )
            psq.append(ps[:])
        for ut in range(2):
            ps = pp.tile([128, 512], f32, tag=f"psv{ut}", name=f"psv{ut}")
            for dt in range(2):
                nc.tensor.matmul(
                    ps[:],
                    lhsT=wv_sb[:, dt * 256 + ut * 128: dt * 256 + ut * 128 + 128],
                    rhs=vT_sb[:, dt * 512: dt * 512 + 512],
                    start=(dt == 0), stop=(dt == 1),
                )
            psv.append(ps)

        projs = {}
        conv_src = {("v", 0): psv[0][:], ("v", 1): psv[1][:],
                    ("q", 0): psq[0], ("q", 1): psq[1]}
        for side, ut, Wd in (("v", 0, 512), ("q", 0, 256),
                             ("v", 1, 512), ("q", 1, 256)):
            projs[(side, ut)] = work.tile(
                [128, Wd], f16, tag=f"pj{side}{ut}", name=f"pj{side}{ut}")

        def emit_conv(side, ut):
            # q0's copy rides ACT's idle head (ACT's first sin is later);
            # the rest stay on DVE
            if (side, ut) == ("q", 0):
                nc.scalar.copy(projs[(side, ut)][:], conv_src[(side, ut)])
            else:
                nc.vector.tensor_copy(projs[(side, ut)][:], conv_src[(side, ut)])

        # ---- arg chains (DVE) / features (ACT) / folds (DVE) / scores (PE)
        streams = [("v", 0, 512), ("q", 0, 256), ("v", 1, 512), ("q", 1, 256)]
        r_t, a_t, z_t, u_t, n_t, s_f, c_f = {}, {}, {}, {}, {}, {}, {}
        for side, ut, Wd in streams:
            key = (side, ut)
            r_t[key] = work.tile([128, K * Wd], f16, tag=f"r{side}{ut}", name=f"r{side}{ut}")
            a_t[key] = work.tile([128, K * Wd], f16, tag=f"a{side}{ut}", name=f"a{side}{ut}")
            z_t[key] = work.tile([128, (K - 1) * Wd], f16, tag=f"z{side}{ut}", name=f"z{side}{ut}")
            u_t[key] = work.tile([128, (K - 1) * Wd], f16, tag=f"u{side}{ut}", name=f"u{side}{ut}")
            n_t[key] = work.tile([128, (K - 1) * Wd], f16, tag=f"n{side}{ut}", name=f"n{side}{ut}")
            s_f[key] = work.tile([128, K * Wd], f16, tag=f"s{side}{ut}", name=f"s{side}{ut}")
            c_f[key] = work.tile([128, K * Wd], f16, tag=f"c{side}{ut}", name=f"c{side}{ut}")
        qws = [work.tile([128, K * 256], f16, tag=f"qws{ut}", name=f"qws{ut}") for ut in range(2)]
        qwc = [work.tile([128, K * 256], f16, tag=f"qwc{ut}", name=f"qwc{ut}") for ut in range(2)]

        # chunks of k-space: k0 alone unblocks ACT/PE early; rest split so
        # the tail releases score matmuls finely.
        CHUNKS = [(0, 1), (1, 3), (3, K)]

        def emit_args(side, ut, c0, c1):
            key = (side, ut)
            Wd = 512 if side == "v" else 256
            proj = projs[key]
            r, a, z, u, n = r_t[key], a_t[key], z_t[key], u_t[key], n_t[key]
            for k in range(c0, c1):
                if k == 0:
                    nc.vector.tensor_scalar_mul(
                        r[:, 0:Wd], proj[:], float(-OM[0] / TWO_PI))
                else:
                    nc.vector.tensor_scalar_mul(
                        z[:, (k - 1) * Wd: k * Wd], proj[:],
                        float(OM[k] / TWO_PI))
            z0, z1 = max(c0 - 1, 0), c1 - 1
            if z1 > z0:
                zs = slice(z0 * Wd, z1 * Wd)
                nc.vector.tensor_scalar_add(u[:, zs], z[:, zs], MAGIC)
                nc.vector.tensor_scalar_sub(n[:, zs], u[:, zs], MAGIC)
                nc.vector.tensor_tensor(
                    r[:, (z0 + 1) * Wd: (z1 + 1) * Wd], n[:, zs], z[:, zs],
                    op=OP.subtract)
            nc.vector.tensor_scalar(
                a[:, c0 * Wd: c1 * Wd].bitcast(u16),
                r[:, c0 * Wd: c1 * Wd].bitcast(u16),
                0x7FFF, None, op0=OP.bitwise_and)

        def emit_feats(side, ut, c0, c1):
            key = (side, ut)
            Wd = 512 if side == "v" else 256
            cs = slice(c0 * Wd, c1 * Wd)
            # r ~= n - z  ->  sin(2pi z) = Sin(-2pi r);  cos = Sin(-2pi|r|+pi/2)
            nc.scalar.activation(s_f[key][:, cs], r_t[key][:, cs], AF.Sin,
                                 scale=-TWO_PI, bias=bias_z[:])
            nc.scalar.activation(c_f[key][:, cs], a_t[key][:, cs], AF.Sin,
                                 scale=-TWO_PI, bias=bias_hpi[:])

        def emit_folds(ut, c0, c1):
            for k in range(c0, c1):
                cs = slice(k * 256, (k + 1) * 256)
                col = fold_sb[:, ut * K + k: ut * K + k + 1]
                nc.vector.tensor_scalar(qws[ut][:, cs], s_f[("q", ut)][:, cs],
                                        col, None, op0=OP.mult)
                nc.vector.tensor_scalar(qwc[ut][:, cs], c_f[("q", ut)][:, cs],
                                        col, None, op0=OP.mult)

        # score regions sc[jt]: [128 j, 256 i]; mask-init; extent i0=64*jt
        sc_banks = [psv[0], psv[1], psq_t[0], psq_t[1]]

        def sc_slice(jt, i0, i1):
            return sc_banks[jt][:, i0:i1]

        def emit_init(jt):
            nc.tensor.matmul(
                sc_slice(jt, 0, 256), lhsT=ident_sb[:],
                rhs=cau_sb[:, jt * 256: jt * 256 + 256],
                start=True, stop=False, skip_group_check=True)
        if DEBUG:
            d_init = spool.tile([128, 256], f32, tag="dinit", name="d_init")
            nc.vector.tensor_copy(d_init[:], sc_slice(0, 0, 256))
            nc.sync.dma_start(dbg_aps["d_init0"], d_init[:])
        reg_left = [2 * 2 * K] * 4

        def emit_scores(ut, c0, c1):
            for k in range(c0, c1):
                for jt in range(4):
                    i0 = 64 * jt
                    for lhs_f, rhs_t in ((c_f[("v", ut)], qws[ut]),
                                         (s_f[("v", ut)], qwc[ut])):
                        reg_left[jt] -= 1
                        nc.tensor.matmul(
                            sc_slice(jt, i0, 256),
                            lhsT=lhs_f[:, k * 512 + jt * 128: k * 512 + jt * 128 + 128],
                            rhs=rhs_t[:, k * 256 + i0: k * 256 + 256],
                            start=False, stop=(reg_left[jt] == 0),
                            skip_group_check=True)

        # esc tiles + zero-fill of non-causal columns
        esc = []
        for jt in range(4):
            t = work.tile([128, 256], f16, tag=f"esc{jt}", name=f"esc{jt}")
            if jt > 0:
                nc.gpsimd.memset(t[:, 0: 64 * jt], 0.0)
            esc.append(t)

        # pipeline: ACT opens on q0-A, q1-A fills the gap until the v
        # projections land; v stream follows; folds trail their feats.
        emit_conv("q", 0)
        emit_args("q", 0, 0, 1)
        emit_feats("q", 0, 0, 1)
        emit_init(2)
        emit_conv("q", 1)
        emit_args("q", 1, 0, 1)
        emit_feats("q", 1, 0, 1)
        emit_init(3)
        emit_conv("v", 0)
        emit_args("v", 0, 0, 1)
        emit_feats("v", 0, 0, 1)
        emit_init(0)
        emit_conv("v", 1)
        emit_args("v", 1, 0, 1)
        emit_feats("v", 1, 0, 1)
        emit_init(1)
        emit_args("v", 0, 1, 3)
        emit_feats("v", 0, 1, 3)
        emit_folds(0, 0, 1)
        emit_scores(0, 0, 1)
        emit_args("q", 0, 1, K)
        emit_feats("q", 0, 1, K)
        emit_folds(1, 0, 1)
        emit_scores(1, 0, 1)
        emit_args("q", 1, 1, K)
        emit_feats("q", 1, 1, K)
        emit_args("v", 1, 1, 3)
        emit_feats("v", 1, 1, 3)
        emit_args("v", 0, 3, K)
        emit_feats("v", 0, 3, K)
        emit_folds(0, 1, 3)
        emit_scores(0, 1, 3)
        emit_folds(1, 1, 3)
        emit_scores(1, 1, 3)
        emit_args("v", 1, 3, K)
        emit_feats("v", 1, 3, K)
        emit_folds(0, 3, K)
        emit_scores(0, 3, K)
        emit_folds(1, 3, K)
        emit_scores(1, 3, K)

        # softmax + context
        ctx_ps = []
        for it in range(2):
            t = pp.tile([128, 257], f32, tag=f"ctx{it}", name=f"ctx{it}")
            ctx_ps.append(t)
        for jt in range(4):
            i0 = 64 * jt
            nc.scalar.activation(esc[jt][:, i0:256], sc_slice(jt, i0, 256),
                                 AF.Exp, bias=bias_sh[:])
            for it in range(2):
                nc.tensor.matmul(
                    ctx_ps[it][:],
                    lhsT=esc[jt][:, it * 128: it * 128 + 128],
                    rhs=val_sb[:, jt * 257: jt * 257 + 257],
                    start=(jt == 0), stop=(jt == 3),
                )
        if DEBUG:
            nc.sync.dma_start(dbg_aps["d_pjv0"], projs[("v", 0)][:])
            nc.sync.dma_start(dbg_aps["d_pjq0"], projs[("q", 0)][:])
            nc.sync.dma_start(dbg_aps["d_sfq0"], s_f[("q", 0)][:])
            nc.sync.dma_start(dbg_aps["d_cfv0"], c_f[("v", 0)][:])
            nc.sync.dma_start(dbg_aps["d_qws0"], qws[0][:])
            nc.sync.dma_start(dbg_aps["d_esc0"], esc[0][:])
            nc.sync.dma_start(dbg_aps["d_esc3"], esc[3][:])
            dbg_ctx = spool.tile([128, 257], f32, tag="dbgctx", name="dbgctx")
            nc.vector.tensor_copy(dbg_ctx[:], ctx_ps[0][:])
            nc.sync.dma_start(dbg_aps["d_ctx0"], dbg_ctx[:])
        for it in range(2):
            rcp = spool.tile([128, 1], f32, tag="rcp", name=f"rcp{it}")
            nc.vector.reciprocal(rcp[:], ctx_ps[it][:, 256:257])
            rq = spool.tile([128, 1], f32, tag="rq", name=f"rq{it}")
            nc.vector.tensor_scalar(rq[:], rcp[:],
                                    fold_sb[:, 2 * K + it: 2 * K + it + 1],
                                    None, op0=OP.mult)
            octx = spool.tile([128, 256], f16, tag="octx", name=f"octx{it}")
            nc.vector.tensor_scalar(octx[:], ctx_ps[it][:, 0:256],
                                    rq[:, 0:1], None, op0=OP.mult)
            (nc.sync if it == 0 else nc.scalar).dma_start(
                ctx_ap[it * 128: it * 128 + 128, :], octx[:])

    nc.compile()
    return nc


_NC_CACHE = {}


def _get_nc():
    if "nc" not in _NC_CACHE:
        _NC_CACHE["nc"] = _build_program()
    return _NC_CACHE["nc"]


def _qsel(h):
    return np.arange(h, S, 2)


def build_in_maps(values, mask, Wq, Wv, Vw):
    values = np.asarray(values, dtype=np.float32)
    mask = np.asarray(mask)
    Wq = np.asarray(Wq, dtype=np.float32)
    Wv = np.asarray(Wv, dtype=np.float32)
    Vw = np.asarray(Vw, dtype=np.float32)

    # weights packed dt-major: [128, 2*256], col block dt -> d rows
    wv_p = np.concatenate([Wv[0:128, :], Wv[128:256, :]], axis=1).astype(np.float16)
    wq_p = np.concatenate([Wq[0:128, :], Wq[128:256, :]], axis=1).astype(np.float16)

    jcol = np.arange(S)
    in_maps = []
    for c in range(N_CORES):
        b, h = divmod(c, 2)
        qs = _qsel(h)
        vb = values[b]  # [512, 256]
        vT = vb.T.astype(np.float16)  # [256 d, 512 j]
        vT_p = np.concatenate([vT[0:128, :], vT[128:256, :]], axis=1)
        qT = vb[qs].T.astype(np.float16)  # [256 d, 256 i]
        qT_p = np.concatenate([qT[0:128, :], qT[128:256, :]], axis=1)
        # values+ones packed jt-major: [128, 4*257]
        vo = np.concatenate(
            [vb.astype(np.float16), np.ones((S, 1), np.float16)], axis=1)
        val_p = np.concatenate([vo[128 * jt: 128 * (jt + 1), :]
                                for jt in range(4)], axis=1)
        # causalT[j, i]: invalid iff qs[i] < j or key j masked out
        inval = (qs[None, :] < jcol[:, None]) | (~mask[b])[:, None]
        cauT = (inval * NEG16).astype(np.float16)  # [512 j, 256 i]
        cau_p = np.concatenate([cauT[128 * jt: 128 * (jt + 1), :]
                                for jt in range(4)], axis=1)
        # fold scalars + per-it query mask columns
        fold = np.zeros((128, 2 * K + 2), np.float32)
        for ut in range(2):
            for k in range(K):
                fold[:, ut * K + k] = BK[k] * Vw[128 * ut: 128 * (ut + 1)]
        qm = mask[b][qs].astype(np.float32)
        fold[:, 2 * K] = qm[0:128]
        fold[:, 2 * K + 1] = qm[128:256]
        in_maps.append({
            "wv": wv_p, "wq": wq_p,
            "vT": np.ascontiguousarray(vT_p),
            "qT": np.ascontiguousarray(qT_p),
            "val": np.ascontiguousarray(val_p),
            "cau": np.ascontiguousarray(cau_p),
            "ident": np.eye(128, dtype=np.float16),
            "fold": fold,
        })
    return in_maps


def kernel(values, mask, Wq, Wv, Vw):
    nc = _get_nc()
    in_maps = build_in_maps(values, mask, Wq, Wv, Vw)
    res = run_bass_kernel_spmd(nc, in_maps, list(range(N_CORES)))

    out = np.empty((B, S, D), dtype=np.float32)
    for c in range(N_CORES):
        b, h = divmod(c, 2)
        out[b, _qsel(h)] = res.results[c]["ctx"].astype(np.float32)
    return out
